# revision 1
# baseline (speedup 1.0000x reference)
"""Trainium2 Bass kernel for nn_DecoderLayer (self-attn -> cross-attn -> FFN).

Distribution: 8 NeuronCores = 4 batches x 2 causal-balanced sequence halves.
Core (b, h) processes 512 query rows of batch b through the entire layer:
half 0 owns row blocks {7,6,1,0} (x128), half 1 owns {5,4,3,2} - descending
block order makes the causally-visible k-tiles of each 512-wide query chunk a
prefix, so one uniform SPMD program serves both halves (masks arrive as data).
No inter-core communication: each core computes K/V projections for the full
sequence itself (small vs. attention cost).

On-device layout is fully transposed (feature dim on SBUF partitions); the
host pre-transposes inputs/weights and re-transposes the output, so the device
does zero transposes. All matmuls run in bf16 (f32 PSUM accumulation); the
residual path stays f32.
"""

import contextlib
import sys
import types

import numpy as np
import ml_dtypes

import concourse.bass as bass
import concourse.mybir as mybir
import concourse.tile as tile

B, L, D, H, DK, DI = 4, 1024, 512, 8, 64, 256
TEMP = float(DK) ** 0.5
EPS = 1e-6
NCORES = 8
KT = D // 128   # 4 feature tiles
LT = L // 128   # 8 sequence tiles
NQ = 512        # own query columns per core
MQ = NQ // 128  # 4 query slots

F32 = mybir.dt.float32
BF16 = mybir.dt.bfloat16
AF = mybir.ActivationFunctionType


# ---------------------------------------------------------------------------
# Workarounds for the walrus build in this container: at most ONE semaphore
# wait per instruction.  Split extra waits onto same-engine NoOps.
# ---------------------------------------------------------------------------

def _patch_drain_split():
    from concourse.vector_clock import ScopedClock

    if getattr(tile.TileContext, "_drain_split_patched", False):
        return

    def _drain_and_barrier(self, tick_clock, wait_clock):
        gc = tick_clock.global_clock
        for idx in range(len(gc)):
            t = gc[idx]
            if t <= 0:
                continue
            req = ScopedClock()
            req.require_at_least(None, idx, t)
            nop_inst = self.nc.sync.nop(nofuse=True, hint=f"drain_wait_{idx}")
            wait_clock.add_sem_waits(nop_inst.ins, req)
        self.nc.sync.drain()
        self.nc.all_engine_barrier()
        assert self.sems is not None
        popped = self.nc._tile_sem_poison_stack.pop()
        assert popped is self._sem_poison
        self.nc.clear_and_free_semaphores(list(self.sems.allocated().values()))
        self.nc.all_engine_barrier()

    tile.TileContext._drain_and_barrier = _drain_and_barrier
    tile.TileContext._drain_split_patched = True


def _split_multi_waits(nc, max_waits=1):
    import bass_rust

    ctr = 0
    for fn in nc.m.functions:
        for blk in fn.blocks:
            changed = False
            new_insts = []
            for inst in blk.instructions:
                si = inst.sync_info
                if si is not None and si.on_wait and len(si.on_wait) > max_waits:
                    waits = list(si.on_wait)
                    for w in waits[:-max_waits]:
                        ctr += 1
                        nop = mybir.InstNoOp(name=f"WSPLIT-{ctr}", ins=[], outs=[])
                        nop.engine = inst.engine
                        nop.sync_info = bass_rust.SyncInfo(on_wait=[w], on_update=[])
                        new_insts.append(nop)
                    inst.sync_info = bass_rust.SyncInfo(
                        on_wait=waits[-max_waits:], on_update=list(si.on_update or [])
                    )
                    changed = True
                new_insts.append(inst)
            if changed:
                blk.instructions = new_insts
    return ctr


_patch_drain_split()


# ---------------------------------------------------------------------------
# Device program
# ---------------------------------------------------------------------------

def _ln_stats(nc, pools, xbf, aux):
    """Emit mean/mean-square stats matmuls (PE) for one layernorm."""
    sb, ps_proj = pools["scratch"], pools["ps_proj"]
    inv_col = aux["inv_col"]
    pmu = ps_proj.tile([1, NQ], F32, name="pmu", tag="proj")
    pmsq = ps_proj.tile([1, NQ], F32, name="pmsq", tag="proj")
    sq = [None] * KT
    for k in range(KT):
        sq[k] = sb.tile([128, NQ], BF16, name="sq", tag=f"sq_{k}", bufs=1)
        nc.vector.tensor_mul(out=sq[k], in0=xbf[k], in1=xbf[k])
    for k in range(KT):
        nc.tensor.matmul(pmu, lhsT=inv_col, rhs=xbf[k], start=(k == 0), stop=(k == KT - 1))
    for k in range(KT):
        nc.tensor.matmul(pmsq, lhsT=inv_col, rhs=sq[k], start=(k == 0), stop=(k == KT - 1))
    return pmu, pmsq


def _ln_rows(nc, pools, pmu, pmsq, aux, dummies=False):
    """Row chain: rstd/mu*rstd as bf16 [1,NQ] rows.  No PE work - emit PE
    matmuls after this point to cover its serial latency.  With dummies=True,
    tiny matmuls chained on intermediates keep the PE HAM-warm when no real
    PE work is available to cover the chain."""
    sb = pools["scratch"]
    ps_proj = pools["ps_proj"]
    W4 = NQ // 128

    def dummy(rhs, bf=False):
        if not dummies:
            return
        if bf:  # [1, n] bf16 row: K=1
            p = ps_proj.tile([64, 128], F32, name="dummy", tag="proj")
            nc.tensor.matmul(p[:, 0:rhs.shape[-1]], lhsT=aux["ones64"], rhs=rhs,
                             start=True, stop=True, skip_group_check=True)
        else:   # [128, n] f32: K=128
            p = ps_proj.tile([1, 128], F32, name="dummy", tag="proj")
            nc.tensor.matmul(p[:, 0:rhs.shape[-1]], lhsT=aux["col32"], rhs=rhs,
                             start=True, stop=True, skip_group_check=True)

    rowpair = sb.tile([1, 2 * NQ], F32, tag="row32", bufs=3)
    nc.scalar.copy(out=rowpair[:, 0:NQ], in_=pmu)
    nc.scalar.copy(out=rowpair[:, NQ:2 * NQ], in_=pmsq)
    mm4 = sb.tile([128, 2, W4], F32, tag="r4", bufs=8)
    nc.sync.dma_start(out=mm4[:, 0, :], in_=rowpair[:, 0:NQ])
    nc.sync.dma_start(out=mm4[:, 1, :], in_=rowpair[:, NQ:2 * NQ])
    mu4 = mm4[:, 0, :]
    msq4 = mm4[:, 1, :]
    dummy(mm4[:, 0, :])
    musq4 = sb.tile([128, W4], F32, tag="r4", bufs=8)
    nc.vector.tensor_mul(out=musq4, in0=mu4, in1=mu4)
    x4 = sb.tile([128, W4], F32, tag="r4", bufs=8)
    nc.vector.tensor_sub(out=x4, in0=msq4, in1=musq4)
    nc.vector.tensor_scalar_add(out=x4, in0=x4, scalar1=EPS)
    # rstd = rsqrt(x4) via DVE-only Newton (seed 1/x; token variance is ~1,
    # quadratic convergence; avoids ACT Sqrt table thrash)
    y = sb.tile([128, W4], F32, tag="r4", bufs=8)
    nc.vector.reciprocal(out=y, in_=x4)
    dummy(x4)
    for it in range(2):
        t = sb.tile([128, W4], F32, tag="r4n", bufs=3)
        nc.vector.tensor_mul(out=t, in0=y, in1=y)
        nc.vector.tensor_mul(out=t, in0=t, in1=x4)
        nc.vector.tensor_scalar(out=t, in0=t, scalar1=-0.5, scalar2=1.5,
                                op0=mybir.AluOpType.mult, op1=mybir.AluOpType.add)
        y2 = sb.tile([128, W4], F32, tag="r4", bufs=8)
        nc.vector.tensor_mul(out=y2, in0=y, in1=t)
        y = y2
        if it == 0:
            dummy(y)
    mr4 = sb.tile([128, W4], F32, tag="r4", bufs=8)
    nc.vector.tensor_mul(out=mr4, in0=mu4, in1=y)
    rrbf = sb.tile([128, 2, W4], BF16, tag="r4b", bufs=4)
    nc.vector.tensor_copy(out=rrbf[:, 0, :], in_=y)
    nc.vector.tensor_copy(out=rrbf[:, 1, :], in_=mr4)
    dummy(mr4)
    rowbf = sb.tile([1, 2 * NQ], BF16, tag="rowbf", bufs=4)
    nc.sync.dma_start(out=rowbf[:, 0:NQ], in_=rrbf[:, 0, :])
    nc.sync.dma_start(out=rowbf[:, NQ:2 * NQ], in_=rrbf[:, 1, :])
    dummy(rowbf[:, 0:128], bf=True)
    return rowbf[:, 0:NQ], rowbf[:, NQ:2 * NQ]


def _ln_xn(nc, pools, x32, rstd_bf, murstd_bf, g_row, b_row, aux):
    """xn[d,l] = x[d,l]*(g[d]*rstd[l]) - (g[d]*(mu*rstd)[l] - b[d])"""
    sb, ps_proj = pools["scratch"], pools["ps_proj"]
    mones = aux["mones"]
    xn = [None] * KT
    for m in range(KT):
        gsl = g_row[:, m * 128:(m + 1) * 128]
        bsl = b_row[:, m * 128:(m + 1) * 128]
        a2 = ps_proj.tile([128, NQ], F32, tag="proj")
        nc.tensor.matmul(a2, lhsT=gsl, rhs=rstd_bf, start=True, stop=True)
        b2 = ps_proj.tile([128, NQ], F32, tag="proj")
        nc.tensor.matmul(b2, lhsT=gsl, rhs=murstd_bf, start=True, stop=False)
        nc.tensor.matmul(b2, lhsT=bsl, rhs=mones, start=False, stop=True,
                         skip_group_check=True)
        tmp = sb.tile([128, NQ], F32, tag="xntmp", bufs=2)
        nc.vector.tensor_mul(out=tmp, in0=x32[m], in1=a2)
        xn[m] = sb.tile([128, NQ], BF16, name="xn", tag=f"xn_{m}", bufs=1)
        nc.vector.tensor_sub(out=xn[m], in0=tmp, in1=b2)
    return xn



def pair_layout(nt_sched):
    """Pack two k-tiles' score columns into one [128,1024] PSUM tile without
    any matmul crossing a 2KB PSUM bank boundary.  Returns per-pair
    (regions, width): regions = list of (t, col_offset, n)."""
    out = []
    for pi in range(LT // 2):
        t0, t1 = 2 * pi, 2 * pi + 1
        n0, n1 = nt_sched[t0], nt_sched[t1]
        regions = []
        off = 0
        if n0 > 0:
            regions.append((t0, 0, n0))
            off = n0
        if n1 > 0:
            o1 = off if off + n1 <= 512 else 512
            regions.append((t1, o1, n1))
            off = o1 + n1
        out.append((regions, off))
    return out


def _xkv_load(nc, pools, xkv):
    act = pools["act"]
    big = act.tile([128, KT, L], BF16, name="xkv_sb", tag="xkv_sb", bufs=2)
    xr = xkv.rearrange("(k p) j -> p k j", p=128)
    nc.sync.dma_start(out=big[:, :, 0:L // 2], in_=xr[:, :, 0:L // 2])
    nc.sync.dma_start(out=big[:, :, L // 2:L], in_=xr[:, :, L // 2:L])
    return [big[:, k, :] for k in range(KT)]


def _k_project(nc, pools, xkv_sb, wk):
    """K projection (independent of the query-side LN, emitted early to keep
    the PE busy through LN chains)."""
    act = pools["act"]
    ps_proj = pools["ps_proj"]
    kT = [None] * KT
    for m in range(KT):
        kT[m] = act.tile([128, L], BF16, name="kT", tag=f"kT_{m}", bufs=2)
        for c in range(L // 512):
            p = ps_proj.tile([128, 512], F32, tag="proj")
            for k in range(KT):
                nc.tensor.matmul(p, lhsT=wk[k][:, m * 128:(m + 1) * 128],
                                 rhs=xkv_sb[k][:, c * 512:(c + 1) * 512],
                                 start=(k == 0), stop=(k == KT - 1))
            if (m + c) % 2 == 0:
                nc.vector.tensor_copy(out=kT[m][:, c * 512:(c + 1) * 512], in_=p)
            else:
                nc.scalar.copy(out=kT[m][:, c * 512:(c + 1) * 512], in_=p)
    return kT


def _kv_fillers(nc, pools, xkv_sb, wk, wv):
    """Closure list computing next-layer K/V one PSUM group at a time -
    popped inside the previous attention's S/AV loop as PE filler work."""
    act = pools["act"]
    ps_proj = pools["ps_proj"]
    kT = [act.tile([128, L], BF16, name="kT", tag=f"kT_{m}", bufs=2)
          for m in range(KT)]
    vv = [act.tile([128, H, 65], BF16, name="vv", tag=f"vv_{t}", bufs=2)
          for t in range(LT)]
    fillers = []

    def kf(m, c):
        def f():
            p = ps_proj.tile([128, 512], F32, tag="proj")
            for k in range(KT):
                nc.tensor.matmul(p, lhsT=wk[k][:, m * 128:(m + 1) * 128],
                                 rhs=xkv_sb[k][:, c * 512:(c + 1) * 512],
                                 start=(k == 0), stop=(k == KT - 1))
            nc.vector.tensor_copy(out=kT[m][:, c * 512:(c + 1) * 512], in_=p)
        return f

    def vf(t):
        def f():
            p = ps_proj.tile([128, 512], F32, tag="proj")
            for k in range(KT):
                nc.tensor.matmul(p, lhsT=xkv_sb[k][:, t * 128:(t + 1) * 128],
                                 rhs=wv[k], start=(k == 0), stop=(k == KT - 1))
            nc.vector.tensor_copy(out=vv[t][:, :, 0:64],
                                  in_=p.rearrange("p (h v) -> p h v", h=H))
            nc.vector.memset(vv[t][:, :, 64:65], 1.0)
        return f

    for m in range(KT):
        for c in range(L // 512):
            fillers.append(kf(m, c))
    for t in range(LT):
        fillers.append(vf(t))
    return kT, vv, fillers


def _v_project(nc, pools, xkv_sb, wv):
    """V [L, H*65] with a ones column per head (softmax denominators)."""
    act = pools["act"]
    ps_proj = pools["ps_proj"]
    vv = [None] * LT
    for t in range(LT):
        p = ps_proj.tile([128, 512], F32, tag="proj")
        for k in range(KT):
            nc.tensor.matmul(p, lhsT=xkv_sb[k][:, t * 128:(t + 1) * 128], rhs=wv[k],
                             start=(k == 0), stop=(k == KT - 1))
        vv[t] = act.tile([128, H, 65], BF16, name="vv", tag=f"vv_{t}", bufs=2)
        nc.vector.tensor_copy(
            out=vv[t][:, :, 0:64],
            in_=p.rearrange("p (h v) -> p h v", h=H),
        )
        nc.vector.memset(vv[t][:, :, 64:65], 1.0)
    return vv



def _q_pre(nc, pools, wq, xin):
    """Q projection on RAW x (LN folded into wq host-side).  Evacuated to
    SBUF f32 immediately so PSUM slots free up."""
    sb = pools["scratch"]
    ps_proj = pools["ps_proj"]
    pre = [None] * KT
    for m in range(KT):
        p = ps_proj.tile([128, NQ], F32, tag="proj")
        for k in range(KT):
            nc.tensor.matmul(p, lhsT=wq[k][:, m * 128:(m + 1) * 128], rhs=xin[k],
                             start=(k == 0), stop=(k == KT - 1))
        pre[m] = sb.tile([128, NQ], F32, name="qpre", tag=f"qpre_{m}", bufs=1)
        if m % 2 == 0:
            nc.vector.tensor_copy(out=pre[m], in_=p)
        else:
            nc.scalar.copy(out=pre[m], in_=p)
    return pre


def _q_correct(nc, pools, pre, rstd_bf, murstd_bf, s_row, b_row, aux):
    """qT[j,l] = rstd[l]*pre[j,l] - murstd[l]*s[j] + b[j]  (bf16 out)."""
    sb = pools["scratch"]
    act = pools["act"]
    ps_s = pools["ps_s"]
    qT = [None] * KT
    for m in range(KT):
        a_b = ps_s.tile([128, NQ], F32, tag="spsum")
        nc.tensor.matmul(a_b, lhsT=aux["ones128"], rhs=rstd_bf, start=True, stop=True)
        b_b = ps_s.tile([128, NQ], F32, tag="spsum")
        nc.tensor.matmul(b_b, lhsT=s_row[:, m * 128:(m + 1) * 128], rhs=murstd_bf,
                         start=True, stop=False)
        nc.tensor.matmul(b_b, lhsT=b_row[:, m * 128:(m + 1) * 128], rhs=aux["mones"],
                         start=False, stop=True, skip_group_check=True)
        t2 = sb.tile([128, NQ], F32, tag="xntmp", bufs=2)
        nc.vector.tensor_mul(out=t2, in0=pre[m], in1=a_b)
        qT[m] = act.tile([128, NQ], BF16, name="qT", tag=f"qT_{m}")
        nc.vector.tensor_sub(out=qT[m], in0=t2, in1=b_b)
    return qT


def _attention(nc, pools, qT, x32, kT, vv, wfc, nt_sched, mask_tiles, aux, fillers=None):
    """One MHA block in transposed layout.  Returns new residual tiles
    (f32) and bf16 copies, consuming xn (queries) + precomputed kT/vv.

    The per-head softmax-normalization tail is software-pipelined: its
    latency chain (PSUM row evac -> [128,4] reshape -> reciprocal -> row
    DMA) is issued near the START of the next head, and its PE/DVE finish
    (broadcast matmul + multiply) near the END of the next head, so the
    in-order engine FIFOs never stall on it."""
    sb = pools["scratch"]
    act = pools["act"]
    ps_proj = pools["ps_proj"]
    ps_s = pools["ps_s"]
    ps_a = pools["ps_a"]
    ppool = pools["ppool"]
    ones64 = aux["ones64"]

    attnT = [act.tile([128, NQ], BF16, name="attnT", tag=f"attnT_{m}") for m in range(KT)]

    def make_tail(h, a_ps):
        state = {}

        def pre():
            den_row = sb.tile([1, NQ], F32, tag="row32", bufs=3)
            nc.scalar.copy(out=den_row, in_=a_ps[64:65, :])
            den4 = sb.tile([128, NQ // 128], F32, tag="r4", bufs=8)
            nc.sync.dma_start(out=den4, in_=den_row)
            r4bf = sb.tile([128, NQ // 128], BF16, tag="r4b", bufs=4)
            with nc.allow_low_precision(reason="softmax denom reciprocal in bf16"):
                nc.vector.reciprocal(out=r4bf, in_=den4)
            recip_bf = sb.tile([1, NQ], BF16, tag="rowbf", bufs=4)
            nc.sync.dma_start(out=recip_bf, in_=r4bf)
            state["recip_bf"] = recip_bf

        def pe():
            rb_ps = ps_s.tile([64, NQ], F32, tag="spsum")
            nc.tensor.matmul(rb_ps, lhsT=ones64, rhs=state["recip_bf"],
                             start=True, stop=True)
            rb_sb = sb.tile([64, NQ], F32, tag="rb", bufs=2)
            nc.vector.tensor_copy(out=rb_sb, in_=rb_ps)
            nc.vector.tensor_mul(
                out=attnT[h // 2][(h % 2) * 64:(h % 2) * 64 + 64, :],
                in0=a_ps[0:64, :], in1=rb_sb,
            )
        return pre, pe

    layout = pair_layout(nt_sched)
    last_pair = max(pi for pi, (rg, _) in enumerate(layout) if rg)
    # out-proj first halves (contraction k=0,1) become late fillers once
    # attnT[0..1] are final (their head tails done by h==5)
    o1 = [None] * KT

    def o_first(m):
        def f():
            p = ps_proj.tile([128, NQ], F32, tag="proj")
            for k in range(2):
                nc.tensor.matmul(p, lhsT=wfc[k][:, m * 128:(m + 1) * 128],
                                 rhs=attnT[k], start=(k == 0), stop=(k == 1))
            o1[m] = sb.tile([128, NQ], F32, name="o1", tag=f"o1_{m}", bufs=1)
            nc.vector.tensor_copy(out=o1[m], in_=p)
        return f

    late = [o_first(m) for m in range(KT)]

    pres, pes = [], []
    for h in range(H):
        kh = kT[h // 2][(h % 2) * 64:(h % 2) * 64 + 64, :]
        qh = qT[h // 2][(h % 2) * 64:(h % 2) * 64 + 64, :]
        a_ps = ps_a.tile([65, NQ], F32, tag="apsum")
        for pi, (regions, width) in enumerate(layout):
            if not regions:
                continue
            if pi == 1 and pres:
                pres.pop(0)()
            if pi == 3 and pes:
                pes.pop(0)()
            for ri, (t, off, n) in enumerate(regions):
                s_ps = ps_s.tile([128, NQ], F32, tag="spsum")
                nc.tensor.matmul(s_ps[:, 0:n],
                                 lhsT=kh[:, t * 128:(t + 1) * 128],
                                 rhs=qh[:, 0:n], start=True, stop=True,
                                 skip_group_check=True)
                p_sb = ppool.tile([128, NQ], BF16, tag="p")
                nc.scalar.activation(out=p_sb[:, 0:n], in_=s_ps[:, 0:n],
                                     func=AF.Exp, scale=1.0 / TEMP)
                if mask_tiles is not None:
                    nc.vector.tensor_mul(out=p_sb[:, 0:n], in0=p_sb[:, 0:n],
                                         in1=mask_tiles[pi][:, off:off + n])
                nc.tensor.matmul(a_ps[:, 0:n], lhsT=vv[t][:, h, :],
                                 rhs=p_sb[:, 0:n],
                                 start=(pi == 0 and ri == 0),
                                 stop=(pi == last_pair and ri == len(regions) - 1),
                                 skip_group_check=True)
            if fillers and 1 <= h <= 2:
                fillers.pop(0)()
            if late and h >= 5 and pi % 2 == 1:
                late.pop(0)()
        p_, e_ = make_tail(h, a_ps)
        pres.append(p_)
        pes.append(e_)
    while late:
        late.pop(0)()

    # Second halves of the output projection (k=2,3).  attnT[2] is final
    # after head 5's tail; attnT[3] needs the last head's tail, so the k=2
    # steps provide PE cover while the final normalization chain drains.
    for f in pres:
        f()
    if fillers:
        for _ in range(2):
            fillers.pop(0)()

    y32 = [None] * KT
    ybf = [None] * KT

    def o_second(m, p):
        t = sb.tile([128, NQ], F32, tag="xntmp", bufs=2)
        nc.vector.tensor_add(out=t, in0=p, in1=o1[m])
        y = act.tile([128, NQ], F32, name="resid", tag=f"resid_{m}", bufs=2)
        nc.vector.tensor_add(out=y, in0=t, in1=x32[m])
        yb = act.tile([128, NQ], BF16, name="xbf", tag=f"xbf_{m}")
        nc.vector.tensor_copy(out=yb, in_=y)
        return y, yb

    p0 = ps_proj.tile([128, NQ], F32, tag="proj")
    p1 = ps_proj.tile([128, NQ], F32, tag="proj")
    nc.tensor.matmul(p0, lhsT=wfc[2][:, 0:128], rhs=attnT[2], start=True, stop=False)
    nc.tensor.matmul(p1, lhsT=wfc[2][:, 128:256], rhs=attnT[2], start=True, stop=False)
    if fillers:
        for _ in range(2):
            fillers.pop(0)()
    for f in pes:
        f()
    nc.tensor.matmul(p0, lhsT=wfc[3][:, 0:128], rhs=attnT[3],
                     start=False, stop=True, skip_group_check=True)
    y32[0], ybf[0] = o_second(0, p0)
    nc.tensor.matmul(p1, lhsT=wfc[3][:, 128:256], rhs=attnT[3],
                     start=False, stop=True, skip_group_check=True)
    y32[1], ybf[1] = o_second(1, p1)
    for m in range(2, KT):
        p = ps_proj.tile([128, NQ], F32, tag="proj")
        for k in range(2, KT):
            nc.tensor.matmul(p, lhsT=wfc[k][:, m * 128:(m + 1) * 128], rhs=attnT[k],
                             start=(k == 2), stop=(k == KT - 1))
        y32[m], ybf[m] = o_second(m, p)
    return y32, ybf


def build_program(nt_sched):
    nc = bass.Bass("TRN2", target_bir_lowering=False, debug=False)

    def din(name, shape, dt=BF16):
        return nc.dram_tensor(name, shape, dt, kind="ExternalInput").ap()

    xq32 = din("xq32", [D, NQ], F32)
    xq16 = din("xq16", [D, NQ])
    xkv = din("xkv", [D, L])
    xenc = din("xenc", [D, L])
    maskd = din("mask", [LT // 2, 128, 2 * NQ])
    w = {n: din(n, [D, D]) for n in
         ["wq_s", "wk_s", "wv_s", "wfc_s", "wq_e", "wk_e", "wv_e", "wfc_e"]}
    w1 = din("w1", [D, DI])
    w2 = din("w2", [DI, D])
    growsd = din("grows", [1, 10, D])
    bvecs = din("bvecs", [128, DI // 128 + KT], F32)
    s1row = din("s1row", [1, DI])
    b1row = din("b1row", [1, DI])
    out_d = nc.dram_tensor("out", [D, NQ], F32, kind="ExternalOutput").ap()

    with tile.TileContext(nc) as tc, contextlib.ExitStack() as ctx:
        pools = {
            "const": ctx.enter_context(tc.tile_pool(name="const", bufs=1)),
            "wpool": ctx.enter_context(tc.tile_pool(name="wpool", bufs=1)),
            "xpool": ctx.enter_context(tc.tile_pool(name="xpool", bufs=1)),
            "act": ctx.enter_context(tc.tile_pool(name="act", bufs=1)),
            "scratch": ctx.enter_context(tc.tile_pool(name="scratch", bufs=2)),
            "ppool": ctx.enter_context(tc.tile_pool(name="ppool", bufs=6)),
            "ps_proj": ctx.enter_context(tc.tile_pool(name="ps_proj", bufs=2, space="PSUM")),
            "ps_s": ctx.enter_context(tc.tile_pool(name="ps_s", bufs=4, space="PSUM")),
            "ps_a": ctx.enter_context(tc.tile_pool(name="ps_a", bufs=2, space="PSUM")),
        }
        const = pools["const"]
        xpool = pools["xpool"]
        wpool = pools["wpool"]
        act = pools["act"]

        # constants
        aux = {}
        aux["inv_col"] = const.tile([128, 1], BF16, name="inv_col")
        nc.vector.memset(aux["inv_col"], 1.0 / D)
        aux["mones"] = const.tile([1, NQ], BF16, name="mones")
        nc.vector.memset(aux["mones"], -1.0)
        aux["eps_t"] = const.tile([128, 1], F32, name="eps_t")
        nc.vector.memset(aux["eps_t"], EPS)
        aux["ones64"] = const.tile([1, 64], BF16, name="ones64")
        nc.vector.memset(aux["ones64"], 1.0)
        aux["ones128"] = const.tile([1, 128], BF16, name="ones128")
        nc.vector.memset(aux["ones128"], 1.0)
        aux["col32"] = const.tile([128, 1], F32, name="col32")
        nc.vector.memset(aux["col32"], 0.0)

        # weights: slf/enc share slots (bufs=1, same tag => enc DMA waits for
        # the self-attention reads to finish)
        def load_w(name, tag):
            big = wpool.tile([128, KT, D], BF16, name=tag, tag=tag)
            nc.sync.dma_start(out=big, in_=w[name].rearrange("(k p) j -> p k j", p=128))
            return [big[:, k, :] for k in range(KT)]

        # earliest DMAs: what the first PE work (self K-proj) needs
        xkv1 = _xkv_load(nc, pools, xkv)
        wk = load_w("wk_s", "wk")
        wv = load_w("wv_s", "wv")
        x32big = xpool.tile([128, KT, NQ], F32, name="xq32t", tag="xq32t")
        nc.sync.dma_start(out=x32big, in_=xq32.rearrange("(k p) j -> p k j", p=128))
        x32 = [x32big[:, m, :] for m in range(KT)]
        x16big = xpool.tile([128, KT, NQ], BF16, name="xq16t", tag="xq16t")
        nc.sync.dma_start(out=x16big, in_=xq16.rearrange("(k p) j -> p k j", p=128))
        xbf = [x16big[:, m, :] for m in range(KT)]
        growbig = const.tile([1, 10, D], BF16, name="growbig")
        nc.sync.dma_start(out=growbig, in_=growsd)
        grows = {n: growbig[:, i, :]
                 for i, n in enumerate(["g_s", "b_s", "g_e", "b_e", "g_f", "b_f",
                                        "sq_s", "bq_s", "sq_e", "bq_e"])}
        bcols = const.tile([128, DI // 128 + KT], F32, name="bcols")
        nc.sync.dma_start(out=bcols, in_=bvecs)
        b1t = [bcols[:, m:m + 1] for m in range(DI // 128)]
        b2t = [bcols[:, DI // 128 + m:DI // 128 + m + 1] for m in range(KT)]

        wq = load_w("wq_s", "wq")
        wfc = load_w("wfc_s", "wfc")

        w1big = wpool.tile([128, KT, DI], BF16, name="w1t", tag="w1t")
        nc.sync.dma_start(out=w1big, in_=w1.rearrange("(k p) j -> p k j", p=128))
        w1t = [w1big[:, k, :] for k in range(KT)]
        w2big = wpool.tile([128, DI // 128, D], BF16, name="w2t", tag="w2t")
        nc.sync.dma_start(out=w2big, in_=w2.rearrange("(k p) j -> p k j", p=128))
        w2t = [w2big[:, k, :] for k in range(DI // 128)]

        # query-side inputs
        mbig = xpool.tile([128, LT // 2, 2 * NQ], BF16, name="maskt", tag="maskt")
        nc.sync.dma_start(out=mbig, in_=maskd.rearrange("t p j -> p t j"))
        mask_tiles = [mbig[:, t, :] for t in range(LT // 2)]

        # ---- self attention ----
        # order: K-proj, LN stats, [LN row chain || V-proj], xn, attention -
        # the V matmuls keep the PE busy through the LN rows' serial latency
        xkv2 = _xkv_load(nc, pools, xenc)
        kT1 = _k_project(nc, pools, xkv1, wk)
        pmu1, pmsq1 = _ln_stats(nc, pools, xbf, aux)
        qpre1 = _q_pre(nc, pools, wq, xbf)
        rstd1, murstd1 = _ln_rows(nc, pools, pmu1, pmsq1, aux)
        vv1 = _v_project(nc, pools, xkv1, wv)
        qT1 = _q_correct(nc, pools, qpre1, rstd1, murstd1,
                         grows["sq_s"], grows["bq_s"], aux)
        # cross-attention weights + K/V fillers: the K/V projection matmul
        # groups are interleaved into the self-attention S/AV loop (which is
        # otherwise ACT-exp-bound) to keep the PE busy and HAM-warm
        wq_e = load_w("wq_e", "wq")
        wk_e = load_w("wk_e", "wk")
        wv_e = load_w("wv_e", "wv")
        wfc_e = load_w("wfc_e", "wfc")
        kT2, vv2, fillers = _kv_fillers(nc, pools, xkv2, wk_e, wv_e)
        x1, x1bf = _attention(nc, pools, qT1, x32, kT1, vv1, wfc,
                              nt_sched, mask_tiles, aux, fillers=fillers)

        # ---- cross attention ----
        pmu2, pmsq2 = _ln_stats(nc, pools, x1bf, aux)
        qpre2 = _q_pre(nc, pools, wq_e, x1bf)
        rstd2, murstd2 = _ln_rows(nc, pools, pmu2, pmsq2, aux)
        while fillers:
            fillers.pop(0)()
        qT2 = _q_correct(nc, pools, qpre2, rstd2, murstd2,
                         grows["sq_e"], grows["bq_e"], aux)
        x2, x2bf = _attention(nc, pools, qT2, x1, kT2, vv2, wfc_e,
                              [NQ] * LT, None, aux)

        # ---- FFN ----
        # LN3 is folded linearly into W1 host-side (w1 = (W1*g).T):
        # h1_in = rstd*pre - murstd*s1 + b1eff, so the h1_pre matmuls are
        # independent of the LN row chain and cover its serial latency.
        s1_sb = const.tile([1, DI], BF16, name="s1_sb")
        nc.sync.dma_start(out=s1_sb, in_=s1row)
        b1_sb = const.tile([1, DI], BF16, name="b1_sb")
        nc.sync.dma_start(out=b1_sb, in_=b1row)
        pmu3, pmsq3 = _ln_stats(nc, pools, x2bf, aux)
        pre_ps = []
        for m in range(DI // 128):
            p = pools["ps_proj"].tile([128, NQ], F32, tag="proj")
            for k in range(KT):
                nc.tensor.matmul(p, lhsT=w1t[k][:, m * 128:(m + 1) * 128], rhs=x2bf[k],
                                 start=(k == 0), stop=(k == KT - 1))
            pre_ps.append(p)
        rstd3, murstd3 = _ln_rows(nc, pools, pmu3, pmsq3, aux, dummies=True)
        relu = []
        for m in range(DI // 128):
            t1 = pools["scratch"].tile([128, NQ], F32, tag="xntmp", bufs=2)
            nc.vector.tensor_copy(out=t1, in_=pre_ps[m])
            a_b = pools["ps_s"].tile([128, NQ], F32, tag="spsum")
            nc.tensor.matmul(a_b, lhsT=aux["ones128"], rhs=rstd3, start=True, stop=True)
            b_b = pools["ps_s"].tile([128, NQ], F32, tag="spsum")
            nc.tensor.matmul(b_b, lhsT=s1_sb[:, m * 128:(m + 1) * 128], rhs=murstd3,
                             start=True, stop=False)
            nc.tensor.matmul(b_b, lhsT=b1_sb[:, m * 128:(m + 1) * 128], rhs=aux["mones"],
                             start=False, stop=True, skip_group_check=True)
            t2 = pools["scratch"].tile([128, NQ], F32, tag="xntmp", bufs=2)
            nc.vector.tensor_mul(out=t2, in0=t1, in1=a_b)
            t3 = pools["scratch"].tile([128, NQ], F32, tag="xntmp", bufs=2)
            nc.vector.tensor_sub(out=t3, in0=t2, in1=b_b)
            r = act.tile([128, NQ], BF16, name="relu", tag=f"relu_{m}")
            nc.scalar.activation(out=r, in_=t3, func=AF.Relu, scale=1.0)
            relu.append(r)
        obig = act.tile([128, KT, NQ], F32, name="obig", tag="obig")
        for m in range(KT):
            p = pools["ps_proj"].tile([128, NQ], F32, tag="proj")
            for k in range(DI // 128):
                nc.tensor.matmul(p, lhsT=w2t[k][:, m * 128:(m + 1) * 128], rhs=relu[k],
                                 start=(k == 0), stop=(k == DI // 128 - 1))
            tmp = pools["scratch"].tile([128, NQ], F32, tag="xntmp", bufs=2)
            nc.scalar.add(out=tmp, in_=p, add=b2t[m])
            nc.vector.tensor_add(out=obig[:, m, :], in0=tmp, in1=x2[m])
            if m == 1:
                nc.sync.dma_start(
                    out=out_d.rearrange("(k p) j -> p k j", p=128)[:, 0:2, :],
                    in_=obig[:, 0:2, :])
        nc.sync.dma_start(out=out_d.rearrange("(k p) j -> p k j", p=128)[:, 2:4, :],
                          in_=obig[:, 2:4, :])

    _split_multi_waits(nc)
    return nc


# ---------------------------------------------------------------------------
# Host side
# ---------------------------------------------------------------------------

_CACHE = {}


def _slot_blocks(half):
    return [7, 6, 1, 0] if half == 0 else [5, 4, 3, 2]


def _qrows(half):
    return np.concatenate([np.arange(b * 128, (b + 1) * 128) for b in _slot_blocks(half)])


def kernel(**inputs):
    dec = np.asarray(inputs["dec_input"], np.float32)
    enc = np.asarray(inputs["enc_output"], np.float32)
    maskin = np.asarray(inputs["slf_attn_mask"])
    mask2d = (maskin[0] != 0)  # [Lq, Lk] bool

    bf = ml_dtypes.bfloat16

    def wT(x):  # [O, D] (or [H,dk,D] stacked) -> transposed bf16 [D, O]
        x = np.asarray(x, np.float32).reshape(-1, x.shape[-1])
        return np.ascontiguousarray(x.T).astype(bf)

    def wq_fold(wkey, gkey, bkey):
        wflat = np.asarray(inputs[wkey], np.float32).reshape(-1, D)     # [hdk, D]
        g = np.asarray(inputs[gkey], np.float32)
        b = np.asarray(inputs[bkey], np.float32)
        wg = wflat * g[None, :]
        sq = wg.sum(axis=1)[None, :].astype(bf)                         # [1, hdk]
        bq = (wflat @ b)[None, :].astype(bf)
        return np.ascontiguousarray(wg.T).astype(bf), sq, bq

    wq_s, sq_s, bq_s = wq_fold("slf_Wq", "slf_ln_g", "slf_ln_b")
    wq_e, sq_e, bq_e = wq_fold("enc_Wq", "enc_ln_g", "enc_ln_b")
    w_t = {
        "wq_s": wq_s, "wk_s": wT(inputs["slf_Wk"]),
        "wv_s": wT(inputs["slf_Wv"]),
        "wfc_s": np.ascontiguousarray(np.asarray(inputs["slf_Wfc"], np.float32).T).astype(bf),
        "wq_e": wq_e, "wk_e": wT(inputs["enc_Wk"]),
        "wv_e": wT(inputs["enc_Wv"]),
        "wfc_e": np.ascontiguousarray(np.asarray(inputs["enc_Wfc"], np.float32).T).astype(bf),
    }
    w1f = np.asarray(inputs["ffn_W1"], np.float32)          # [DI, D]
    g_f = np.asarray(inputs["ffn_ln_g"], np.float32)
    b_lnf = np.asarray(inputs["ffn_ln_b"], np.float32)
    w1g = w1f * g_f[None, :]
    w1 = np.ascontiguousarray(w1g.T).astype(bf)
    s1row = (w1g.sum(axis=1))[None, :].astype(bf)           # [1, DI]
    b1row = (w1f @ b_lnf + np.asarray(inputs["ffn_b1"], np.float32))[None, :].astype(bf)
    w2 = np.ascontiguousarray(np.asarray(inputs["ffn_W2"], np.float32).T).astype(bf)
    grows_arr = np.concatenate([
        np.stack([np.asarray(inputs[k], np.float32).astype(bf) for k in
                  ["slf_ln_g", "slf_ln_b", "enc_ln_g", "enc_ln_b",
                   "ffn_ln_g", "ffn_ln_b"]]),
        np.stack([sq_s[0], bq_s[0], sq_e[0], bq_e[0]]),
    ])[None]                                              # [1, 10, D]
    b1 = np.asarray(inputs["ffn_b1"], np.float32)
    b2 = np.asarray(inputs["ffn_b2"], np.float32)
    bvecs = np.stack(
        [b1[0:128], b1[128:256]] + [b2[m * 128:(m + 1) * 128] for m in range(KT)],
        axis=1).astype(np.float32)                        # [128, 6]

    # per-core mask tiles + uniform prefix schedule
    half_masks = []
    nt_sched = [0] * LT
    for half in range(2):
        rowsq = _qrows(half)
        m = mask2d[rowsq, :]          # [NQ, Lk]
        tiles = np.zeros((LT, 128, NQ), np.float32)
        for t in range(LT):
            blk = m[:, t * 128:(t + 1) * 128]     # [NQ, 128]
            tiles[t] = blk.T.astype(np.float32)
            for s in range(MQ):
                if blk[s * 128:(s + 1) * 128, :].any():
                    nt_sched[t] = max(nt_sched[t], (s + 1) * 128)
        half_masks.append(tiles)
    nt_sched[0] = NQ  # first tile initializes the full accumulator
    nt_key = tuple(nt_sched)

    # pack per-pair mask tiles matching the device's pair_layout
    layout = pair_layout(nt_sched)
    core_masks = []
    for half in range(2):
        packed = np.zeros((LT // 2, 128, 2 * NQ), np.float32)
        for pi, (regions, _w) in enumerate(layout):
            for (t, off, n) in regions:
                packed[pi][:, off:off + n] = half_masks[half][t][:, 0:n]
        core_masks.append(packed.astype(bf))

    if nt_key not in _CACHE:
        _CACHE[nt_key] = build_program(list(nt_key))
    nc = _CACHE[nt_key]

    in_maps = []
    for c in range(NCORES):
        b, half = divmod(c, 2)
        rowsq = _qrows(half)
        decT = np.ascontiguousarray(dec[b].T)          # [D, L] f32
        encT = np.ascontiguousarray(enc[b].T)
        xq32 = np.ascontiguousarray(decT[:, rowsq])
        in_maps.append({
            "xq32": xq32,
            "xq16": xq32.astype(bf),
            "xkv": decT.astype(bf),
            "xenc": encT.astype(bf),
            "mask": core_masks[half],
            "w1": w1, "w2": w2,
            "s1row": s1row, "b1row": b1row,
            "bvecs": bvecs, "grows": grows_arr,
            **w_t,
        })

    from concourse.bass_utils import run_bass_kernel_spmd

    res = run_bass_kernel_spmd(nc, in_maps, core_ids=list(range(NCORES)))
    globals()["_LAST_RESULT"] = res

    out = np.empty((B, L, D), np.float32)
    for c in range(NCORES):
        b, half = divmod(c, 2)
        out[b, _qrows(half), :] = res.results[c]["out"].T
    return out



# revision 2
# speedup vs baseline: 1.2431x; 1.2431x over previous
"""Trainium2 Bass kernel for nn_DecoderLayer (self-attn -> cross-attn -> FFN).

Distribution: 8 NeuronCores = 4 batches x 2 causal-balanced sequence halves.
Core (b, h) processes 512 query rows of batch b through the entire layer:
half 0 owns row blocks {7,5,2,0} (x128), half 1 owns {6,4,3,1} - descending
block order makes the causally-visible k-tiles of each query chunk a prefix,
so one uniform SPMD program serves both halves (masks arrive as data).
No inter-core communication: each core computes K/V projections for the full
sequence itself.

On-device layout is fully transposed (feature dim on SBUF partitions); the
host pre-transposes inputs/weights and re-transposes the output.  All matmuls
run in bf16 (f32 PSUM accumulation); the residual path stays f32.

LayerNorm row statistics are computed directly in broadcast form: the stats
matmul uses an all-1/D [128,128] stationary tile, so mean / mean-square land
replicated across all 128 partitions and the rsqrt Newton iteration runs as
full-width DVE ops - no DMA reshape round trips, no PE broadcast matmuls.
Softmax denominators ride the AV matmul: V carries 64 extra all-ones columns,
so PSUM rows 64:128 hold the denominator replicated 64-wide and the per-head
normalization is one DVE reciprocal + one multiply.
"""

import contextlib

import numpy as np
import ml_dtypes

import concourse.bass as bass
import concourse.mybir as mybir
import concourse.tile as tile

B, L, D, H, DK, DI = 4, 1024, 512, 8, 64, 256
TEMP = float(DK) ** 0.5
NCORES = 8
KT = D // 128   # 4 feature tiles
LT = L // 128   # 8 sequence tiles
NQ = 512        # own query columns per core
MQ = NQ // 128  # 4 query slots

F32 = mybir.dt.float32
BF16 = mybir.dt.bfloat16
AF = mybir.ActivationFunctionType
ALU = None


# ---------------------------------------------------------------------------
# Workarounds for the walrus build in this container: at most ONE semaphore
# wait per instruction.  Split extra waits onto same-engine NoOps.
# ---------------------------------------------------------------------------

def _patch_drain_split():
    from concourse.vector_clock import ScopedClock

    if getattr(tile.TileContext, "_drain_split_patched", False):
        return

    def _drain_and_barrier(self, tick_clock, wait_clock):
        gc = tick_clock.global_clock
        for idx in range(len(gc)):
            t = gc[idx]
            if t <= 0:
                continue
            req = ScopedClock()
            req.require_at_least(None, idx, t)
            nop_inst = self.nc.sync.nop(nofuse=True, hint=f"drain_wait_{idx}")
            wait_clock.add_sem_waits(nop_inst.ins, req)
        self.nc.sync.drain()
        self.nc.all_engine_barrier()
        assert self.sems is not None
        popped = self.nc._tile_sem_poison_stack.pop()
        assert popped is self._sem_poison
        self.nc.clear_and_free_semaphores(list(self.sems.allocated().values()))
        self.nc.all_engine_barrier()

    tile.TileContext._drain_and_barrier = _drain_and_barrier
    tile.TileContext._drain_split_patched = True


def _split_multi_waits(nc, max_waits=1):
    import bass_rust

    ctr = 0
    for fn in nc.m.functions:
        for blk in fn.blocks:
            changed = False
            new_insts = []
            for inst in blk.instructions:
                si = inst.sync_info
                if si is not None and si.on_wait and len(si.on_wait) > max_waits:
                    waits = list(si.on_wait)
                    for w in waits[:-max_waits]:
                        ctr += 1
                        nop = mybir.InstNoOp(name=f"WSPLIT-{ctr}", ins=[], outs=[])
                        nop.engine = inst.engine
                        nop.sync_info = bass_rust.SyncInfo(on_wait=[w], on_update=[])
                        new_insts.append(nop)
                    inst.sync_info = bass_rust.SyncInfo(
                        on_wait=waits[-max_waits:], on_update=list(si.on_update or [])
                    )
                    changed = True
                new_insts.append(inst)
            if changed:
                blk.instructions = new_insts
    return ctr


_patch_drain_split()


# ---------------------------------------------------------------------------
# Device program
# ---------------------------------------------------------------------------

def _ln_stats(nc, pools, xbf, aux):
    """mu / mean-square stats, broadcast across all 128 partitions."""
    sb, ps_s = pools["scratch"], pools["ps_s"]
    invD = aux["invD"]
    pmu = ps_s.tile([128, NQ], F32, name="pmu", tag="spsum")
    pmsq = ps_s.tile([128, NQ], F32, name="pmsq", tag="spsum")
    sq = [None] * KT
    for k in range(KT):
        sq[k] = sb.tile([128, NQ], BF16, name="sq", tag=f"sq_{k}", bufs=1)
        nc.vector.tensor_mul(out=sq[k], in0=xbf[k], in1=xbf[k])
    for k in range(KT):
        nc.tensor.matmul(pmu, lhsT=invD, rhs=xbf[k], start=(k == 0), stop=(k == KT - 1))
    for k in range(KT):
        nc.tensor.matmul(pmsq, lhsT=invD, rhs=sq[k], start=(k == 0), stop=(k == KT - 1))
    return pmu, pmsq


def _ln_rows(nc, pools, pmu, pmsq):
    """rstd / mu*rstd as broadcast [128,NQ] bf16 tiles.  ACT does the mu^2
    (square is resident in the exp table set), DVE does a bf16 Newton rsqrt.
    No DMA, no PE work."""
    sb = pools["scratch"]
    from concourse.alu_op_type import AluOpType
    musq = sb.tile([128, NQ], F32, tag="lnt", bufs=3)
    nc.scalar.activation(out=musq, in_=pmu, func=AF.Square, scale=1.0)
    x = sb.tile([128, NQ], BF16, tag="lnb", bufs=6)
    nc.vector.tensor_sub(out=x, in0=pmsq, in1=musq)
    y = sb.tile([128, NQ], BF16, tag="lnb", bufs=6)
    with nc.allow_low_precision(reason="layernorm rsqrt newton in bf16"):
        nc.vector.reciprocal(out=y, in_=x)
        for _ in range(2):
            t = sb.tile([128, NQ], BF16, tag="lnb", bufs=6)
            nc.vector.tensor_mul(out=t, in0=y, in1=y)
            nc.vector.tensor_mul(out=t, in0=t, in1=x)
            nc.vector.tensor_scalar(out=t, in0=t, scalar1=-0.5, scalar2=1.5,
                                    op0=AluOpType.mult, op1=AluOpType.add)
            y2 = sb.tile([128, NQ], BF16, tag="lnb", bufs=6)
            nc.vector.tensor_mul(out=y2, in0=y, in1=t)
            y = y2
        mr = sb.tile([128, NQ], BF16, tag="lnb", bufs=6)
        nc.vector.tensor_mul(out=mr, in0=pmu, in1=y)
    return y, mr


def _q_correct(nc, pools, pre, rstd_b, murstd_b, s_cols, b_cols):
    """qT[j,l] = rstd[l]*pre[j,l] - (murstd[l]*s[j] - b[j]) - all DVE."""
    from concourse.alu_op_type import AluOpType
    sb = pools["scratch"]
    act = pools["act"]
    qT = [None] * KT
    for m in range(KT):
        bb = sb.tile([128, NQ], BF16, tag="lnb", bufs=6)
        nc.vector.tensor_scalar(out=bb, in0=murstd_b, scalar1=s_cols[m],
                                scalar2=b_cols[m], op0=AluOpType.mult,
                                op1=AluOpType.subtract)
        t2 = sb.tile([128, NQ], F32, tag="xntmp", bufs=2)
        nc.vector.tensor_mul(out=t2, in0=pre[m], in1=rstd_b)
        qT[m] = act.tile([128, NQ], BF16, name="qT", tag=f"qT_{m}")
        nc.vector.tensor_sub(out=qT[m], in0=t2, in1=bb)
    return qT


def pair_layout(nt_sched):
    """Pack two k-tiles' score columns into one [128,1024] PSUM tile without
    any matmul crossing a 2KB PSUM bank boundary."""
    out = []
    for pi in range(LT // 2):
        t0, t1 = 2 * pi, 2 * pi + 1
        n0, n1 = nt_sched[t0], nt_sched[t1]
        regions = []
        off = 0
        if n0 > 0:
            regions.append((t0, 0, n0))
            off = n0
        if n1 > 0:
            o1 = off if off + n1 <= 512 else 512
            regions.append((t1, o1, n1))
            off = o1 + n1
        out.append((regions, off))
    return out


def _contig_runs(regions):
    """Group regions into maximal contiguous column runs [(start, end, [t...])]."""
    runs = []
    for (t, off, n) in regions:
        if runs and runs[-1][1] == off:
            runs[-1][1] = off + n
            runs[-1][2].append(t)
        else:
            runs.append([off, off + n, [t]])
    return runs


def _xkv_load(nc, pools, xkv):
    act = pools["act"]
    big = act.tile([128, KT, L], BF16, name="xkv_sb", tag="xkv_sb", bufs=2)
    xr = xkv.rearrange("(k p) j -> p k j", p=128)
    nc.sync.dma_start(out=big[:, :, 0:L // 2], in_=xr[:, :, 0:L // 2])
    nc.sync.dma_start(out=big[:, :, L // 2:L], in_=xr[:, :, L // 2:L])
    return [big[:, k, :] for k in range(KT)]


def _k_project(nc, pools, xkv_sb, wk):
    act = pools["act"]
    ps_proj = pools["ps_proj"]
    kT = [None] * KT
    for m in range(KT):
        kT[m] = act.tile([128, L], BF16, name="kT", tag=f"kT_{m}", bufs=2)
        for c in range(L // 512):
            p = ps_proj.tile([128, 512], F32, tag="proj")
            for k in range(KT):
                nc.tensor.matmul(p, lhsT=wk[k][:, m * 128:(m + 1) * 128],
                                 rhs=xkv_sb[k][:, c * 512:(c + 1) * 512],
                                 start=(k == 0), stop=(k == KT - 1))
            if (m + c) % 2 == 0:
                nc.vector.tensor_copy(out=kT[m][:, c * 512:(c + 1) * 512], in_=p)
            else:
                nc.scalar.copy(out=kT[m][:, c * 512:(c + 1) * 512], in_=p)
    return kT


def _v_fill(nc, pools, xkv_sb, wv, vv, t):
    """One V-projection PSUM group into vv[t][:, :, 0:64]."""
    ps_proj = pools["ps_proj"]
    p = ps_proj.tile([128, 512], F32, tag="proj")
    for k in range(KT):
        nc.tensor.matmul(p, lhsT=xkv_sb[k][:, t * 128:(t + 1) * 128], rhs=wv[k],
                         start=(k == 0), stop=(k == KT - 1))
    nc.vector.tensor_copy(out=vv[t][:, :, 0:64],
                          in_=p.rearrange("p (h v) -> p h v", h=H))


def _kv_fillers(nc, pools, xkv_sb, wk, wv, vv2):
    """Closure list computing next-layer K/V one PSUM group at a time."""
    act = pools["act"]
    ps_proj = pools["ps_proj"]
    kT = [act.tile([128, L], BF16, name="kT", tag=f"kT_{m}", bufs=2)
          for m in range(KT)]
    fillers = []

    def kf(m, c):
        def f():
            p = ps_proj.tile([128, 512], F32, tag="proj")
            for k in range(KT):
                nc.tensor.matmul(p, lhsT=wk[k][:, m * 128:(m + 1) * 128],
                                 rhs=xkv_sb[k][:, c * 512:(c + 1) * 512],
                                 start=(k == 0), stop=(k == KT - 1))
            nc.vector.tensor_copy(out=kT[m][:, c * 512:(c + 1) * 512], in_=p)
        return f

    for m in range(KT):
        for c in range(L // 512):
            fillers.append(kf(m, c))
    for t in range(LT):
        fillers.append(lambda t=t: _v_fill(nc, pools, xkv_sb, wv, vv2, t))
    return kT, fillers


def _q_pre(nc, pools, wq, xin):
    sb = pools["scratch"]
    ps_proj = pools["ps_proj"]
    pre = [None] * KT
    for m in range(KT):
        p = ps_proj.tile([128, NQ], F32, tag="proj")
        for k in range(KT):
            nc.tensor.matmul(p, lhsT=wq[k][:, m * 128:(m + 1) * 128], rhs=xin[k],
                             start=(k == 0), stop=(k == KT - 1))
        pre[m] = sb.tile([128, NQ], F32, name="qpre", tag=f"qpre_{m}", bufs=1)
        if m % 2 == 0:
            nc.vector.tensor_copy(out=pre[m], in_=p)
        else:
            nc.scalar.copy(out=pre[m], in_=p)
    return pre


def _attention(nc, pools, qT, x32, kT, vv, wfc, nt_sched, mask_tiles, fillers=None):
    """One MHA block.  a_ps rows 0:64 = per-head AV, rows 64:128 = softmax
    denominator broadcast 64-wide (via the ones half of vv).  Per-head tail
    is one DVE reciprocal + one multiply, issued one head late."""
    sb = pools["scratch"]
    act = pools["act"]
    ps_proj = pools["ps_proj"]
    ps_s = pools["ps_s"]
    ps_a = pools["ps_a"]
    ppool = pools["ppool"]

    attnT = [act.tile([128, NQ], BF16, name="attnT", tag=f"attnT_{m}") for m in range(KT)]

    def make_tail(h, a_ps):
        def f():
            rb = sb.tile([64, NQ], BF16, name="rb", tag="rb", bufs=3)
            with nc.allow_low_precision(reason="softmax denom reciprocal in bf16"):
                nc.vector.reciprocal(out=rb, in_=a_ps[64:128, :])
            nc.vector.tensor_mul(
                out=attnT[h // 2][(h % 2) * 64:(h % 2) * 64 + 64, :],
                in0=a_ps[0:64, :], in1=rb,
            )
        return f

    layout = pair_layout(nt_sched)
    last_pair = max(pi for pi, (rg, _) in enumerate(layout) if rg)
    o1 = [None] * KT

    def o_first(m):
        def f():
            p = ps_proj.tile([128, NQ], F32, tag="proj")
            for k in range(2):
                nc.tensor.matmul(p, lhsT=wfc[k][:, m * 128:(m + 1) * 128],
                                 rhs=attnT[k], start=(k == 0), stop=(k == 1))
            o1[m] = sb.tile([128, NQ], F32, name="o1", tag=f"o1_{m}", bufs=1)
            nc.vector.tensor_copy(out=o1[m], in_=p)
        return f

    late = [o_first(m) for m in range(KT)]

    tails = []
    for h in range(H):
        kh = kT[h // 2][(h % 2) * 64:(h % 2) * 64 + 64, :]
        qh = qT[h // 2][(h % 2) * 64:(h % 2) * 64 + 64, :]
        a_ps = ps_a.tile([128, NQ], F32, tag="apsum")
        for pi, (regions, width) in enumerate(layout):
            if not regions:
                continue
            if pi == 1 and tails:
                tails.pop(0)()
            s_ps = ps_s.tile([128, 2 * NQ], F32, tag="spsum")
            p_sb = ppool.tile([128, 2 * NQ], BF16, tag="p")
            for (t, off, n) in regions:
                nc.tensor.matmul(s_ps[:, off:off + n],
                                 lhsT=kh[:, t * 128:(t + 1) * 128],
                                 rhs=qh[:, 0:n], start=True, stop=True,
                                 skip_group_check=True)
            for (lo, hi, _ts) in _contig_runs(regions):
                nc.scalar.activation(out=p_sb[:, lo:hi], in_=s_ps[:, lo:hi],
                                     func=AF.Exp, scale=1.0 / TEMP)
                if mask_tiles is not None:
                    nc.vector.tensor_mul(out=p_sb[:, lo:hi], in0=p_sb[:, lo:hi],
                                         in1=mask_tiles[pi][:, lo:hi])
            for ri, (t, off, n) in enumerate(regions):
                nc.tensor.matmul(a_ps[:, 0:n], lhsT=vv[t][:, h, :],
                                 rhs=p_sb[:, off:off + n],
                                 start=(pi == 0 and ri == 0),
                                 stop=(pi == last_pair and ri == len(regions) - 1),
                                 skip_group_check=True)
            if fillers and 1 <= h <= 2:
                fillers.pop(0)()
            if late and h >= 5 and pi % 2 == 1:
                late.pop(0)()
        tails.append(make_tail(h, a_ps))
    while late:
        late.pop(0)()

    # Flush remaining head tails with PE filler between them, then the
    # second halves (k=2,3) of the output projection.
    if fillers:
        fillers.pop(0)()
    while tails:
        tails.pop(0)()
        if fillers:
            fillers.pop(0)()

    y32 = [None] * KT
    ybf = [None] * KT

    def o_second(m, p):
        t = sb.tile([128, NQ], F32, tag="xntmp", bufs=2)
        nc.vector.tensor_add(out=t, in0=p, in1=o1[m])
        y = act.tile([128, NQ], F32, name="resid", tag=f"resid_{m}", bufs=2)
        nc.vector.tensor_add(out=y, in0=t, in1=x32[m])
        yb = act.tile([128, NQ], BF16, name="xbf", tag=f"xbf_{m}")
        nc.scalar.copy(out=yb, in_=y)
        return y, yb

    for m in range(KT):
        p = ps_proj.tile([128, NQ], F32, tag="proj")
        for k in range(2, KT):
            nc.tensor.matmul(p, lhsT=wfc[k][:, m * 128:(m + 1) * 128], rhs=attnT[k],
                             start=(k == 2), stop=(k == KT - 1))
        y32[m], ybf[m] = o_second(m, p)
    return y32, ybf


def build_program(nt_sched):
    from concourse.alu_op_type import AluOpType
    nc = bass.Bass("TRN2", target_bir_lowering=False, debug=False)

    def din(name, shape, dt=BF16):
        return nc.dram_tensor(name, shape, dt, kind="ExternalInput").ap()

    xq16 = din("xq16", [D, NQ])
    xq32 = din("xq32", [D, NQ], F32)
    xkv = din("xkv", [D, L])
    xenc = din("xenc", [D, L])
    maskd = din("mask", [LT // 2, 128, 2 * NQ])
    w = {n: din(n, [D, D]) for n in
         ["wq_s", "wk_s", "wv_s", "wfc_s", "wq_e", "wk_e", "wv_e", "wfc_e"]}
    w1 = din("w1", [D, DI])
    w2 = din("w2", [DI, D])
    scolsd = din("scols", [128, 20], F32)
    bvecs = din("bvecs", [128, DI // 128 + KT], F32)
    out_d = nc.dram_tensor("out", [D, NQ], F32, kind="ExternalOutput").ap()

    with tile.TileContext(nc) as tc, contextlib.ExitStack() as ctx:
        pools = {
            "const": ctx.enter_context(tc.tile_pool(name="const", bufs=1)),
            "wpool": ctx.enter_context(tc.tile_pool(name="wpool", bufs=1)),
            "xpool": ctx.enter_context(tc.tile_pool(name="xpool", bufs=1)),
            "act": ctx.enter_context(tc.tile_pool(name="act", bufs=1)),
            "scratch": ctx.enter_context(tc.tile_pool(name="scratch", bufs=2)),
            "ppool": ctx.enter_context(tc.tile_pool(name="ppool", bufs=4)),
            "ps_proj": ctx.enter_context(tc.tile_pool(name="ps_proj", bufs=2, space="PSUM")),
            "ps_s": ctx.enter_context(tc.tile_pool(name="ps_s", bufs=2, space="PSUM")),
            "ps_a": ctx.enter_context(tc.tile_pool(name="ps_a", bufs=2, space="PSUM")),
        }
        const = pools["const"]
        xpool = pools["xpool"]
        wpool = pools["wpool"]
        act = pools["act"]

        aux = {}
        aux["invD"] = const.tile([128, 128], BF16, name="invD")
        nc.vector.memset(aux["invD"], 1.0 / D)

        def load_w(name, tag):
            big = wpool.tile([128, KT, D], BF16, name=tag, tag=tag)
            nc.sync.dma_start(out=big, in_=w[name].rearrange("(k p) j -> p k j", p=128))
            return [big[:, k, :] for k in range(KT)]

        # earliest DMAs in need order: stats1 needs xq16; K-proj needs wk+xkv
        x16big = xpool.tile([128, KT, NQ], BF16, name="xq16t", tag="xq16t")
        nc.sync.dma_start(out=x16big, in_=xq16.rearrange("(k p) j -> p k j", p=128))
        xbf = [x16big[:, m, :] for m in range(KT)]
        wk = load_w("wk_s", "wk")
        xkv1 = _xkv_load(nc, pools, xkv)
        wq = load_w("wq_s", "wq")
        wv = load_w("wv_s", "wv")
        mbig = xpool.tile([128, LT // 2, 2 * NQ], BF16, name="maskt", tag="maskt")
        nc.sync.dma_start(out=mbig, in_=maskd.rearrange("t p j -> p t j"))
        mask_tiles = [mbig[:, t, :] for t in range(LT // 2)]
        x32big = xpool.tile([128, KT, NQ], F32, name="xq32t", tag="xq32t")
        nc.sync.dma_start(out=x32big, in_=xq32.rearrange("(k p) j -> p k j", p=128))
        x32 = [x32big[:, m, :] for m in range(KT)]
        wfc = load_w("wfc_s", "wfc")
        scols_t = const.tile([128, 20], F32, name="scols_t")
        nc.sync.dma_start(out=scols_t, in_=scolsd)
        sq_s = [scols_t[:, m:m + 1] for m in range(KT)]
        bq_s = [scols_t[:, 4 + m:5 + m] for m in range(KT)]
        sq_e = [scols_t[:, 8 + m:9 + m] for m in range(KT)]
        bq_e = [scols_t[:, 12 + m:13 + m] for m in range(KT)]
        s1c = [scols_t[:, 16 + m:17 + m] for m in range(DI // 128)]
        b1c = [scols_t[:, 18 + m:19 + m] for m in range(DI // 128)]
        bcols = const.tile([128, DI // 128 + KT], F32, name="bcols")
        nc.sync.dma_start(out=bcols, in_=bvecs)
        b2t = [bcols[:, DI // 128 + m:DI // 128 + m + 1] for m in range(KT)]
        xkv2 = _xkv_load(nc, pools, xenc)
        wq_e = load_w("wq_e", "wq2")
        wk_e = load_w("wk_e", "wk2")
        wv_e = load_w("wv_e", "wv2")
        wfc_e = load_w("wfc_e", "wfc2")
        w1big = wpool.tile([128, KT, DI], BF16, name="w1t", tag="w1t")
        nc.sync.dma_start(out=w1big, in_=w1.rearrange("(k p) j -> p k j", p=128))
        w1t = [w1big[:, k, :] for k in range(KT)]
        w2big = wpool.tile([128, DI // 128, D], BF16, name="w2t", tag="w2t")
        nc.sync.dma_start(out=w2big, in_=w2.rearrange("(k p) j -> p k j", p=128))
        w2t = [w2big[:, k, :] for k in range(DI // 128)]

        # V tiles with the ones half pre-set (softmax denominator broadcast)
        vv1 = [act.tile([128, H, 128], BF16, name="vv1", tag=f"vv1_{t}")
               for t in range(LT)]
        vv2 = [act.tile([128, H, 128], BF16, name="vv2", tag=f"vv2_{t}")
               for t in range(LT)]
        for t in range(LT):
            nc.vector.memset(vv1[t][:, :, 64:128], 1.0)
            nc.vector.memset(vv2[t][:, :, 64:128], 1.0)

        # ---- self attention ----
        pmu1, pmsq1 = _ln_stats(nc, pools, xbf, aux)
        kT1 = _k_project(nc, pools, xkv1, wk)
        rstd1, murstd1 = _ln_rows(nc, pools, pmu1, pmsq1)
        qpre1 = _q_pre(nc, pools, wq, xbf)
        for t in range(LT):
            _v_fill(nc, pools, xkv1, wv, vv1, t)
        qT1 = _q_correct(nc, pools, qpre1, rstd1, murstd1, sq_s, bq_s)
        kT2, fillers = _kv_fillers(nc, pools, xkv2, wk_e, wv_e, vv2)
        x1, x1bf = _attention(nc, pools, qT1, x32, kT1, vv1, wfc,
                              nt_sched, mask_tiles, fillers=fillers)

        # ---- cross attention ----
        pmu2, pmsq2 = _ln_stats(nc, pools, x1bf, aux)
        qpre2 = _q_pre(nc, pools, wq_e, x1bf)
        rstd2, murstd2 = _ln_rows(nc, pools, pmu2, pmsq2)
        while fillers:
            fillers.pop(0)()
        qT2 = _q_correct(nc, pools, qpre2, rstd2, murstd2, sq_e, bq_e)
        x2, x2bf = _attention(nc, pools, qT2, x1, kT2, vv2, wfc_e,
                              [NQ] * LT, None)

        # ---- FFN ----
        pmu3, pmsq3 = _ln_stats(nc, pools, x2bf, aux)
        pre_ps = []
        for m in range(DI // 128):
            p = pools["ps_proj"].tile([128, NQ], F32, tag="proj")
            for k in range(KT):
                nc.tensor.matmul(p, lhsT=w1t[k][:, m * 128:(m + 1) * 128], rhs=x2bf[k],
                                 start=(k == 0), stop=(k == KT - 1))
            pre_ps.append(p)
        rstd3, murstd3 = _ln_rows(nc, pools, pmu3, pmsq3)
        relu = []
        for m in range(DI // 128):
            bb = pools["scratch"].tile([128, NQ], BF16, tag="lnb", bufs=6)
            nc.vector.tensor_scalar(out=bb, in0=murstd3, scalar1=s1c[m],
                                    scalar2=b1c[m], op0=AluOpType.mult,
                                    op1=AluOpType.subtract)
            t2 = pools["scratch"].tile([128, NQ], F32, tag="xntmp", bufs=2)
            nc.vector.tensor_mul(out=t2, in0=pre_ps[m], in1=rstd3)
            t3 = pools["scratch"].tile([128, NQ], F32, tag="xntmp", bufs=2)
            nc.vector.tensor_sub(out=t3, in0=t2, in1=bb)
            r = act.tile([128, NQ], BF16, name="relu", tag=f"relu_{m}")
            nc.scalar.activation(out=r, in_=t3, func=AF.Relu, scale=1.0)
            relu.append(r)
        obig = act.tile([128, KT, NQ], F32, name="obig", tag="obig")
        for m in range(KT):
            p = pools["ps_proj"].tile([128, NQ], F32, tag="proj")
            for k in range(DI // 128):
                nc.tensor.matmul(p, lhsT=w2t[k][:, m * 128:(m + 1) * 128], rhs=relu[k],
                                 start=(k == 0), stop=(k == DI // 128 - 1))
            tmp = pools["scratch"].tile([128, NQ], F32, tag="xntmp", bufs=2)
            nc.scalar.add(out=tmp, in_=p, add=b2t[m])
            nc.vector.tensor_add(out=obig[:, m, :], in0=tmp, in1=x2[m])
            if m == 1:
                nc.sync.dma_start(
                    out=out_d.rearrange("(k p) j -> p k j", p=128)[:, 0:2, :],
                    in_=obig[:, 0:2, :])
        nc.sync.dma_start(out=out_d.rearrange("(k p) j -> p k j", p=128)[:, 2:4, :],
                          in_=obig[:, 2:4, :])

    _split_multi_waits(nc)
    return nc


# ---------------------------------------------------------------------------
# Host side
# ---------------------------------------------------------------------------

_CACHE = {}


def _slot_blocks(half):
    return [7, 5, 2, 0] if half == 0 else [6, 4, 3, 1]


def _qrows(half):
    return np.concatenate([np.arange(b * 128, (b + 1) * 128) for b in _slot_blocks(half)])


def kernel(**inputs):
    dec = np.asarray(inputs["dec_input"], np.float32)
    enc = np.asarray(inputs["enc_output"], np.float32)
    maskin = np.asarray(inputs["slf_attn_mask"])
    mask2d = (maskin[0] != 0)  # [Lq, Lk] bool

    bf = ml_dtypes.bfloat16

    def wT(x):
        x = np.asarray(x, np.float32).reshape(-1, x.shape[-1])
        return np.ascontiguousarray(x.T).astype(bf)

    def wq_fold(wkey, gkey, bkey):
        wflat = np.asarray(inputs[wkey], np.float32).reshape(-1, D)     # [hdk, D]
        g = np.asarray(inputs[gkey], np.float32)
        b = np.asarray(inputs[bkey], np.float32)
        wg = wflat * g[None, :]
        s = wg.sum(axis=1)                                              # [hdk]
        bq = wflat @ b
        return np.ascontiguousarray(wg.T).astype(bf), s, bq

    wq_s, s_s, b_s = wq_fold("slf_Wq", "slf_ln_g", "slf_ln_b")
    wq_e, s_e, b_e = wq_fold("enc_Wq", "enc_ln_g", "enc_ln_b")
    w_t = {
        "wq_s": wq_s, "wk_s": wT(inputs["slf_Wk"]),
        "wv_s": wT(inputs["slf_Wv"]),
        "wfc_s": np.ascontiguousarray(np.asarray(inputs["slf_Wfc"], np.float32).T).astype(bf),
        "wq_e": wq_e, "wk_e": wT(inputs["enc_Wk"]),
        "wv_e": wT(inputs["enc_Wv"]),
        "wfc_e": np.ascontiguousarray(np.asarray(inputs["enc_Wfc"], np.float32).T).astype(bf),
    }
    w1f = np.asarray(inputs["ffn_W1"], np.float32)          # [DI, D]
    g_f = np.asarray(inputs["ffn_ln_g"], np.float32)
    b_lnf = np.asarray(inputs["ffn_ln_b"], np.float32)
    w1g = w1f * g_f[None, :]
    w1 = np.ascontiguousarray(w1g.T).astype(bf)
    s1 = w1g.sum(axis=1)                                    # [DI]
    b1eff = w1f @ b_lnf + np.asarray(inputs["ffn_b1"], np.float32)
    w2 = np.ascontiguousarray(np.asarray(inputs["ffn_W2"], np.float32).T).astype(bf)

    # per-partition scalar columns: [128, 20]
    def cols(v, n):
        return np.stack([v[m * 128:(m + 1) * 128] for m in range(n)], axis=1)

    scols = np.concatenate([
        cols(s_s, KT), cols(b_s, KT), cols(s_e, KT), cols(b_e, KT),
        cols(s1, DI // 128), cols(b1eff, DI // 128),
    ], axis=1).astype(np.float32)                           # [128, 20]

    b1 = np.asarray(inputs["ffn_b1"], np.float32)
    b2 = np.asarray(inputs["ffn_b2"], np.float32)
    bvecs = np.stack(
        [b1[0:128], b1[128:256]] + [b2[m * 128:(m + 1) * 128] for m in range(KT)],
        axis=1).astype(np.float32)                          # [128, 6]

    # per-core mask tiles + uniform prefix schedule
    half_masks = []
    nt_sched = [0] * LT
    for half in range(2):
        rowsq = _qrows(half)
        m = mask2d[rowsq, :]
        tiles = np.zeros((LT, 128, NQ), np.float32)
        for t in range(LT):
            blk = m[:, t * 128:(t + 1) * 128]
            tiles[t] = blk.T.astype(np.float32)
            for s in range(MQ):
                if blk[s * 128:(s + 1) * 128, :].any():
                    nt_sched[t] = max(nt_sched[t], (s + 1) * 128)
        half_masks.append(tiles)
    nt_sched[0] = NQ
    nt_key = tuple(nt_sched)

    layout = pair_layout(nt_sched)
    core_masks = []
    for half in range(2):
        packed = np.zeros((LT // 2, 128, 2 * NQ), np.float32)
        for pi, (regions, _w) in enumerate(layout):
            for (t, off, n) in regions:
                packed[pi][:, off:off + n] = half_masks[half][t][:, 0:n]
        core_masks.append(packed.astype(bf))

    if nt_key not in _CACHE:
        _CACHE[nt_key] = build_program(list(nt_key))
    nc = _CACHE[nt_key]

    in_maps = []
    for c in range(NCORES):
        b, half = divmod(c, 2)
        rowsq = _qrows(half)
        decT = np.ascontiguousarray(dec[b].T)          # [D, L] f32
        encT = np.ascontiguousarray(enc[b].T)
        xq32 = np.ascontiguousarray(decT[:, rowsq])
        in_maps.append({
            "xq32": xq32,
            "xq16": xq32.astype(bf),
            "xkv": decT.astype(bf),
            "xenc": encT.astype(bf),
            "mask": core_masks[half],
            "w1": w1, "w2": w2,
            "scols": scols,
            "bvecs": bvecs,
            **w_t,
        })

    from concourse.bass_utils import run_bass_kernel_spmd

    res = run_bass_kernel_spmd(nc, in_maps, core_ids=list(range(NCORES)))
    globals()["_LAST_RESULT"] = res

    out = np.empty((B, L, D), np.float32)
    for c in range(NCORES):
        b, half = divmod(c, 2)
        out[b, _qrows(half), :] = res.results[c]["out"].T
    return out


# revision 15
# speedup vs baseline: 1.5429x; 1.2412x over previous
"""Trainium2 Bass kernel for nn_DecoderLayer (self-attn -> cross-attn -> FFN).

Distribution: 8 NeuronCores = 4 batches x 2 causal-balanced sequence halves.
Core (b, h) processes 512 query rows of batch b through the entire layer:
half 0 owns row blocks {7,5,2,0} (x128), half 1 owns {6,4,3,1} - descending
block order makes the causally-visible k-tiles of each query chunk a prefix,
so one uniform SPMD program serves both halves (masks arrive as data).
No inter-core communication: each core computes K/V projections for the full
sequence itself.

On-device layout is fully transposed (feature dim on SBUF partitions); the
host pre-transposes inputs/weights and re-transposes the output.  All matmuls
run in bf16 (f32 PSUM accumulation); the residual path stays f32.

LayerNorm row statistics are computed directly in broadcast form: the stats
matmul uses an all-1/D [128,128] stationary tile, so mean / mean-square land
replicated across all 128 partitions and the rsqrt Newton iteration runs as
full-width DVE ops - no DMA reshape round trips, no PE broadcast matmuls.
Softmax denominators ride the AV matmul: V carries 64 extra all-ones columns,
so PSUM rows 64:128 hold the denominator replicated 64-wide and the per-head
normalization is one DVE reciprocal + one multiply.
"""

import contextlib

import numpy as np
import ml_dtypes

import concourse.bass as bass
import concourse.mybir as mybir
import concourse.tile as tile

B, L, D, H, DK, DI = 4, 1024, 512, 8, 64, 256
TEMP = float(DK) ** 0.5
NCORES = 8
KT = D // 128   # 4 feature tiles
LT = L // 128   # 8 sequence tiles
NQ = 512        # own query columns per core
MQ = NQ // 128  # 4 query slots

F32 = mybir.dt.float32
BF16 = mybir.dt.bfloat16
AF = mybir.ActivationFunctionType
ALU = None


# ---------------------------------------------------------------------------
# Workarounds for the walrus build in this container: at most ONE semaphore
# wait per instruction.  Split extra waits onto same-engine NoOps.
# ---------------------------------------------------------------------------

def _patch_drain_split():
    from concourse.vector_clock import ScopedClock

    if getattr(tile.TileContext, "_drain_split_patched", False):
        return

    def _drain_and_barrier(self, tick_clock, wait_clock):
        gc = tick_clock.global_clock
        for idx in range(len(gc)):
            t = gc[idx]
            if t <= 0:
                continue
            req = ScopedClock()
            req.require_at_least(None, idx, t)
            nop_inst = self.nc.sync.nop(nofuse=True, hint=f"drain_wait_{idx}")
            wait_clock.add_sem_waits(nop_inst.ins, req)
        self.nc.sync.drain()
        self.nc.all_engine_barrier()
        assert self.sems is not None
        popped = self.nc._tile_sem_poison_stack.pop()
        assert popped is self._sem_poison
        self.nc.clear_and_free_semaphores(list(self.sems.allocated().values()))
        self.nc.all_engine_barrier()

    tile.TileContext._drain_and_barrier = _drain_and_barrier
    tile.TileContext._drain_split_patched = True


def _split_multi_waits(nc, max_waits=1):
    import bass_rust

    ctr = 0
    for fn in nc.m.functions:
        for blk in fn.blocks:
            changed = False
            new_insts = []
            for inst in blk.instructions:
                si = inst.sync_info
                if si is not None and si.on_wait and len(si.on_wait) > max_waits:
                    waits = list(si.on_wait)
                    for w in waits[:-max_waits]:
                        ctr += 1
                        nop = mybir.InstNoOp(name=f"WSPLIT-{ctr}", ins=[], outs=[])
                        nop.engine = inst.engine
                        nop.sync_info = bass_rust.SyncInfo(on_wait=[w], on_update=[])
                        new_insts.append(nop)
                    inst.sync_info = bass_rust.SyncInfo(
                        on_wait=waits[-max_waits:], on_update=list(si.on_update or [])
                    )
                    changed = True
                new_insts.append(inst)
            if changed:
                blk.instructions = new_insts
    return ctr


_patch_drain_split()


# ---------------------------------------------------------------------------
# Device program
# ---------------------------------------------------------------------------

def _ln_stats(nc, pools, xbf, aux):
    """mu / mean-square stats, broadcast across all 128 partitions."""
    sb, ps_s = pools["scratch"], pools["ps_s"]
    invD = aux["invD"]
    pmu = ps_s.tile([128, NQ], F32, name="pmu", tag="spsum")
    pmsq = ps_s.tile([128, NQ], F32, name="pmsq", tag="spsum")
    sq = [None] * KT
    for k in range(KT):
        sq[k] = sb.tile([128, NQ], BF16, name="sq", tag=f"sq_{k}", bufs=1)
        nc.vector.tensor_mul(out=sq[k], in0=xbf[k], in1=xbf[k])
    for k in range(KT):
        nc.tensor.matmul(pmu, lhsT=invD, rhs=xbf[k], start=(k == 0), stop=(k == KT - 1))
    for k in range(KT):
        nc.tensor.matmul(pmsq, lhsT=invD, rhs=sq[k], start=(k == 0), stop=(k == KT - 1))
    return pmu, pmsq


def _ln_rows(nc, pools, pmu, pmsq):
    """rstd / mu*rstd as broadcast [128,NQ] bf16 tiles.  All serial steps on
    ACT: square, then rsqrt(var) = exp(-0.5*ln(var)) - ln and exp are
    resident together in the natural_log_exp table set.  DVE does only the
    variance subtract and the mu*rstd multiply."""
    sb = pools["scratch"]
    musq = sb.tile([128, NQ], F32, tag="lnt", bufs=2)
    nc.scalar.activation(out=musq, in_=pmu, func=AF.Square, scale=1.0)
    var = sb.tile([128, NQ], F32, tag="lnt", bufs=2)
    nc.vector.tensor_sub(out=var, in0=pmsq, in1=musq)
    lg = sb.tile([128, NQ], F32, tag="lnt", bufs=2)
    nc.scalar.activation(out=lg, in_=var, func=AF.Ln, scale=1.0)
    y = sb.tile([128, NQ], BF16, tag="lnb", bufs=6)
    nc.scalar.activation(out=y, in_=lg, func=AF.Exp, scale=-0.5)
    mr = sb.tile([128, NQ], BF16, tag="lnb", bufs=6)
    with nc.allow_low_precision(reason="layernorm rows in bf16"):
        nc.vector.tensor_mul(out=mr, in0=pmu, in1=y)
    return y, mr


def _q_correct(nc, pools, pre, rstd_b, murstd_b, s_cols, b_cols):
    """qT[j,l] = rstd[l]*pre[j,l] - (murstd[l]*s[j] - b[j]) - all DVE."""
    from concourse.alu_op_type import AluOpType
    sb = pools["scratch"]
    act = pools["act"]
    qT = [None] * KT
    for m in range(KT):
        bb = sb.tile([128, NQ], BF16, tag="lnb", bufs=6)
        nc.vector.tensor_scalar(out=bb, in0=murstd_b, scalar1=s_cols[m],
                                scalar2=b_cols[m], op0=AluOpType.mult,
                                op1=AluOpType.subtract)
        t2 = sb.tile([128, NQ], BF16, tag="lnb", bufs=6)
        with nc.allow_low_precision(reason="q in bf16 anyway"):
            nc.vector.tensor_mul(out=t2, in0=pre[m], in1=rstd_b)
        qT[m] = act.tile([128, NQ], BF16, name="qT", tag=f"qT_{m}")
        nc.vector.tensor_sub(out=qT[m], in0=t2, in1=bb)
    return qT


def pair_layout(nt_sched):
    """Pack two k-tiles' score columns into one [128,1024] PSUM tile without
    any matmul crossing a 2KB PSUM bank boundary."""
    out = []
    for pi in range(LT // 2):
        t0, t1 = 2 * pi, 2 * pi + 1
        n0, n1 = nt_sched[t0], nt_sched[t1]
        regions = []
        off = 0
        if n0 > 0:
            regions.append((t0, 0, n0))
            off = n0
        if n1 > 0:
            o1 = off if off + n1 <= 512 else 512
            regions.append((t1, o1, n1))
            off = o1 + n1
        out.append((regions, off))
    return out


def _contig_runs(regions):
    """Group regions into maximal contiguous column runs [(start, end, [t...])]."""
    runs = []
    for (t, off, n) in regions:
        if runs and runs[-1][1] == off:
            runs[-1][1] = off + n
            runs[-1][2].append(t)
        else:
            runs.append([off, off + n, [t]])
    return runs


def _xkv_load(nc, pools, xkv):
    act = pools["act"]
    big = act.tile([128, KT, L], BF16, name="xkv_sb", tag="xkv_sb", bufs=2)
    xr = xkv.rearrange("(k p) j -> p k j", p=128)
    nc.sync.dma_start(out=big[:, :, 0:L // 2], in_=xr[:, :, 0:L // 2])
    nc.sync.dma_start(out=big[:, :, L // 2:L], in_=xr[:, :, L // 2:L])
    return [big[:, k, :] for k in range(KT)]


def _k_project(nc, pools, xkv_sb, wk):
    act = pools["act"]
    ps_proj = pools["ps_proj"]
    kT = [None] * KT
    for m in range(KT):
        kT[m] = act.tile([128, L], BF16, name="kT", tag=f"kT_{m}", bufs=2)
        for c in range(L // 512):
            p = ps_proj.tile([128, 512], F32, tag="proj")
            for k in range(KT):
                nc.tensor.matmul(p, lhsT=wk[k][:, m * 128:(m + 1) * 128],
                                 rhs=xkv_sb[k][:, c * 512:(c + 1) * 512],
                                 start=(k == 0), stop=(k == KT - 1))
            if (m + c) % 2 == 0:
                nc.vector.tensor_copy(out=kT[m][:, c * 512:(c + 1) * 512], in_=p)
            else:
                nc.scalar.copy(out=kT[m][:, c * 512:(c + 1) * 512], in_=p)
    return kT


def _v_fill(nc, pools, xkv_sb, wv, vv, t):
    """One V-projection PSUM group into vv[t][:, :, 0:64] (evac on ACT)."""
    ps_proj = pools["ps_proj"]
    p = ps_proj.tile([128, 512], F32, tag="proj")
    for k in range(KT):
        nc.tensor.matmul(p, lhsT=xkv_sb[k][:, t * 128:(t + 1) * 128], rhs=wv[k],
                         start=(k == 0), stop=(k == KT - 1))
    nc.scalar.copy(out=vv[t][:, :, 0:64],
                   in_=p.rearrange("p (h v) -> p h v", h=H))


def _kv_fillers(nc, pools, xkv_sb, wk, wv, vv2):
    """Closure list computing next-layer K/V one PSUM group at a time."""
    act = pools["act"]
    ps_proj = pools["ps_proj"]
    kT = [act.tile([128, L], BF16, name="kT", tag=f"kT_{m}", bufs=2)
          for m in range(KT)]
    fillers = []

    def kf(m, c):
        def f():
            p = ps_proj.tile([128, 512], F32, tag="proj")
            for k in range(KT):
                nc.tensor.matmul(p, lhsT=wk[k][:, m * 128:(m + 1) * 128],
                                 rhs=xkv_sb[k][:, c * 512:(c + 1) * 512],
                                 start=(k == 0), stop=(k == KT - 1))
            if (m + c) % 2 == 0:
                nc.vector.tensor_copy(out=kT[m][:, c * 512:(c + 1) * 512], in_=p)
            else:
                nc.scalar.copy(out=kT[m][:, c * 512:(c + 1) * 512], in_=p)
        return f

    for m in range(KT):
        for c in range(L // 512):
            fillers.append(kf(m, c))
    for t in range(LT):
        fillers.append(lambda t=t: _v_fill(nc, pools, xkv_sb, wv, vv2, t))
    return kT, fillers


def _q_pre(nc, pools, wq, xin):
    sb = pools["scratch"]
    ps_proj = pools["ps_proj"]
    pre = [None] * KT
    for m in range(KT):
        p = ps_proj.tile([128, NQ], F32, tag="proj")
        for k in range(KT):
            nc.tensor.matmul(p, lhsT=wq[k][:, m * 128:(m + 1) * 128], rhs=xin[k],
                             start=(k == 0), stop=(k == KT - 1))
        pre[m] = sb.tile([128, NQ], F32, name="qpre", tag=f"qpre_{m}", bufs=1)
        if m % 2 == 0:
            nc.vector.tensor_copy(out=pre[m], in_=p)
        else:
            nc.scalar.copy(out=pre[m], in_=p)
    return pre


def _attention(nc, pools, qT, x32, kT, vv, wfc, nt_sched, mask_tiles, fillers=None,
               post_m=None):
    """One MHA block.  a_ps rows 0:64 = per-head AV, rows 64:128 = softmax
    denominator broadcast 64-wide (via the ones half of vv).  Per-head tail
    is one DVE reciprocal + one multiply, issued one head late."""
    sb = pools["scratch"]
    act = pools["act"]
    ps_proj = pools["ps_proj"]
    ps_s = pools["ps_s"]
    ps_a = pools["ps_a"]
    ppool = pools["ppool"]

    attnT = [act.tile([128, NQ], BF16, name="attnT", tag=f"attnT_{m}") for m in range(KT)]

    def make_tail(h, a_ps):
        # 1/den = exp(-ln(den)) on ACT (both tables resident); DVE only
        # does the final multiply.
        def f():
            lg = sb.tile([64, NQ], F32, name="lg", tag="lg", bufs=2)
            nc.scalar.activation(out=lg, in_=a_ps[64:128, :], func=AF.Ln, scale=1.0)
            rb = sb.tile([64, NQ], BF16, name="rb", tag="rb", bufs=2)
            nc.scalar.activation(out=rb, in_=lg, func=AF.Exp, scale=-1.0)
            nc.vector.tensor_mul(
                out=attnT[h // 2][(h % 2) * 64:(h % 2) * 64 + 64, :],
                in0=a_ps[0:64, :], in1=rb,
            )
        return f

    layout = pair_layout(nt_sched)
    last_pair = max(pi for pi, (rg, _) in enumerate(layout) if rg)
    o1 = [None] * KT

    def o_first(m):
        def f():
            p = ps_proj.tile([128, NQ], F32, tag="proj")
            for k in range(2):
                nc.tensor.matmul(p, lhsT=wfc[k][:, m * 128:(m + 1) * 128],
                                 rhs=attnT[k], start=(k == 0), stop=(k == 1))
            o1[m] = sb.tile([128, NQ], F32, name="o1", tag=f"o1_{m}", bufs=1)
            if m % 2 == 0:
                nc.vector.tensor_copy(out=o1[m], in_=p)
            else:
                nc.scalar.copy(out=o1[m], in_=p)
        return f

    late = [o_first(m) for m in range(KT)]

    tails = []
    for h in range(H):
        kh = kT[h // 2][(h % 2) * 64:(h % 2) * 64 + 64, :]
        qh = qT[h // 2][(h % 2) * 64:(h % 2) * 64 + 64, :]
        a_ps = ps_a.tile([128, NQ], F32, tag="apsum")
        for pi, (regions, width) in enumerate(layout):
            if not regions:
                continue
            if pi == 1 and tails:
                tails.pop(0)()
            s_ps = ps_s.tile([128, 2 * NQ], F32, tag="spsum")
            p_sb = ppool.tile([128, 2 * NQ], BF16, tag="p")
            for (t, off, n) in regions:
                nc.tensor.matmul(s_ps[:, off:off + n],
                                 lhsT=kh[:, t * 128:(t + 1) * 128],
                                 rhs=qh[:, 0:n], start=True, stop=True,
                                 skip_group_check=True)
            for (lo, hi, _ts) in _contig_runs(regions):
                nc.scalar.activation(out=p_sb[:, lo:hi], in_=s_ps[:, lo:hi],
                                     func=AF.Exp, scale=1.0 / TEMP)
                if mask_tiles is not None:
                    nc.vector.tensor_mul(out=p_sb[:, lo:hi], in0=p_sb[:, lo:hi],
                                         in1=mask_tiles[pi][:, lo:hi])
            for ri, (t, off, n) in enumerate(regions):
                nc.tensor.matmul(a_ps[:, 0:n], lhsT=vv[t][:, h, :],
                                 rhs=p_sb[:, off:off + n],
                                 start=(pi == 0 and ri == 0),
                                 stop=(pi == last_pair and ri == len(regions) - 1),
                                 skip_group_check=True)
            if fillers and 1 <= h <= 2:
                fillers.pop(0)()
            if late and h >= 5 and pi % 2 == 1:
                late.pop(0)()
        tails.append(make_tail(h, a_ps))
    while late:
        late.pop(0)()

    # Flush remaining head tails with PE filler between them, then the
    # second halves (k=2,3) of the output projection.
    if fillers:
        fillers.pop(0)()
    while tails:
        tails.pop(0)()
        if fillers:
            fillers.pop(0)()

    y32 = [None] * KT
    ybf = [None] * KT

    def o_second(m, p):
        t = sb.tile([128, NQ], F32, tag="xntmp", bufs=2)
        nc.vector.tensor_add(out=t, in0=p, in1=o1[m])
        y = act.tile([128, NQ], F32, name="resid", tag=f"resid_{m}", bufs=2)
        nc.vector.tensor_add(out=y, in0=t, in1=x32[m])
        yb = act.tile([128, NQ], BF16, name="xbf", tag=f"xbf_{m}")
        nc.scalar.copy(out=yb, in_=y)
        return y, yb

    for m in range(KT):
        p = ps_proj.tile([128, NQ], F32, tag="proj")
        for k in range(2, KT):
            nc.tensor.matmul(p, lhsT=wfc[k][:, m * 128:(m + 1) * 128], rhs=attnT[k],
                             start=(k == 2), stop=(k == KT - 1))
        y32[m], ybf[m] = o_second(m, p)
        if post_m is not None:
            post_m(m, ybf[m])
    return y32, ybf


def build_program(nt_sched):
    from concourse.alu_op_type import AluOpType
    nc = bass.Bass("TRN2", target_bir_lowering=False, debug=False)

    def din(name, shape, dt=BF16):
        return nc.dram_tensor(name, shape, dt, kind="ExternalInput").ap()

    xq16 = din("xq16", [D, NQ])
    xq32 = din("xq32", [D, NQ], F32)
    xkv = din("xkv", [D, L])
    xenc = din("xenc", [D, L])
    maskd = din("mask", [LT // 2, 128, 2 * NQ])
    w = {n: din(n, [D, D]) for n in
         ["wq_s", "wk_s", "wv_s", "wfc_s", "wq_e", "wk_e", "wv_e", "wfc_e"]}
    w1 = din("w1", [D, DI])
    w2 = din("w2", [DI, D])
    scolsd = din("scols", [128, 20], F32)
    bvecs = din("bvecs", [128, DI // 128 + KT], F32)
    out_d = nc.dram_tensor("out", [D, NQ], F32, kind="ExternalOutput").ap()

    with tile.TileContext(nc) as tc, contextlib.ExitStack() as ctx:
        pools = {
            "const": ctx.enter_context(tc.tile_pool(name="const", bufs=1)),
            "wpool": ctx.enter_context(tc.tile_pool(name="wpool", bufs=1)),
            "xpool": ctx.enter_context(tc.tile_pool(name="xpool", bufs=1)),
            "act": ctx.enter_context(tc.tile_pool(name="act", bufs=1)),
            "scratch": ctx.enter_context(tc.tile_pool(name="scratch", bufs=2)),
            "ppool": ctx.enter_context(tc.tile_pool(name="ppool", bufs=4)),
            "ps_proj": ctx.enter_context(tc.tile_pool(name="ps_proj", bufs=2, space="PSUM")),
            "ps_s": ctx.enter_context(tc.tile_pool(name="ps_s", bufs=2, space="PSUM")),
            "ps_a": ctx.enter_context(tc.tile_pool(name="ps_a", bufs=2, space="PSUM")),
        }
        const = pools["const"]
        xpool = pools["xpool"]
        wpool = pools["wpool"]
        act = pools["act"]

        aux = {}
        aux["invD"] = const.tile([128, 128], BF16, name="invD")
        nc.vector.memset(aux["invD"], 1.0 / D)

        def load_w(name, tag):
            big = wpool.tile([128, KT, D], BF16, name=tag, tag=tag)
            nc.sync.dma_start(out=big, in_=w[name].rearrange("(k p) j -> p k j", p=128))
            return [big[:, k, :] for k in range(KT)]

        # earliest DMAs in need order: stats1 needs xq16; K-proj needs wk+xkv
        x16big = xpool.tile([128, KT, NQ], BF16, name="xq16t", tag="xq16t")
        nc.sync.dma_start(out=x16big, in_=xq16.rearrange("(k p) j -> p k j", p=128))
        xbf = [x16big[:, m, :] for m in range(KT)]
        wk = load_w("wk_s", "wk")
        xkv1 = _xkv_load(nc, pools, xkv)
        wq = load_w("wq_s", "wq")
        wv = load_w("wv_s", "wv")
        mbig = xpool.tile([128, LT // 2, 2 * NQ], BF16, name="maskt", tag="maskt")
        nc.sync.dma_start(out=mbig, in_=maskd.rearrange("t p j -> p t j"))
        mask_tiles = [mbig[:, t, :] for t in range(LT // 2)]
        x32big = xpool.tile([128, KT, NQ], F32, name="xq32t", tag="xq32t")
        nc.sync.dma_start(out=x32big, in_=xq32.rearrange("(k p) j -> p k j", p=128))
        x32 = [x32big[:, m, :] for m in range(KT)]
        wfc = load_w("wfc_s", "wfc")
        scols_t = const.tile([128, 20], F32, name="scols_t")
        nc.sync.dma_start(out=scols_t, in_=scolsd)
        sq_s = [scols_t[:, m:m + 1] for m in range(KT)]
        bq_s = [scols_t[:, 4 + m:5 + m] for m in range(KT)]
        sq_e = [scols_t[:, 8 + m:9 + m] for m in range(KT)]
        bq_e = [scols_t[:, 12 + m:13 + m] for m in range(KT)]
        s1c = [scols_t[:, 16 + m:17 + m] for m in range(DI // 128)]
        b1c = [scols_t[:, 18 + m:19 + m] for m in range(DI // 128)]
        bcols = const.tile([128, DI // 128 + KT], F32, name="bcols")
        nc.sync.dma_start(out=bcols, in_=bvecs)
        b2t = [bcols[:, DI // 128 + m:DI // 128 + m + 1] for m in range(KT)]
        xkv2 = _xkv_load(nc, pools, xenc)
        wq_e = load_w("wq_e", "wq2")
        wk_e = load_w("wk_e", "wk2")
        wv_e = load_w("wv_e", "wv2")
        wfc_e = load_w("wfc_e", "wfc2")
        w1big = wpool.tile([128, KT, DI], BF16, name="w1t", tag="w1t")
        nc.sync.dma_start(out=w1big, in_=w1.rearrange("(k p) j -> p k j", p=128))
        w1t = [w1big[:, k, :] for k in range(KT)]
        w2big = wpool.tile([128, DI // 128, D], BF16, name="w2t", tag="w2t")
        nc.sync.dma_start(out=w2big, in_=w2.rearrange("(k p) j -> p k j", p=128))
        w2t = [w2big[:, k, :] for k in range(DI // 128)]

        # V tiles with the ones half pre-set (softmax denominator broadcast)
        vv1 = [act.tile([128, H, 128], BF16, name="vv1", tag=f"vv1_{t}")
               for t in range(LT)]
        vv2 = [act.tile([128, H, 128], BF16, name="vv2", tag=f"vv2_{t}")
               for t in range(LT)]
        for t in range(LT):
            nc.gpsimd.memset(vv1[t][:, :, 64:128], 1.0)
            nc.gpsimd.memset(vv2[t][:, :, 64:128], 1.0)

        # ---- self attention ----
        pmu1, pmsq1 = _ln_stats(nc, pools, xbf, aux)
        kT1 = _k_project(nc, pools, xkv1, wk)
        rstd1, murstd1 = _ln_rows(nc, pools, pmu1, pmsq1)
        qpre1 = _q_pre(nc, pools, wq, xbf)
        for t in range(LT):
            _v_fill(nc, pools, xkv1, wv, vv1, t)
        qT1 = _q_correct(nc, pools, qpre1, rstd1, murstd1, sq_s, bq_s)
        kT2, fillers = _kv_fillers(nc, pools, xkv2, wk_e, wv_e, vv2)
        x1, x1bf = _attention(nc, pools, qT1, x32, kT1, vv1, wfc,
                              nt_sched, mask_tiles, fillers=fillers)

        # ---- cross attention ----
        pmu2, pmsq2 = _ln_stats(nc, pools, x1bf, aux)
        qpre2 = _q_pre(nc, pools, wq_e, x1bf)
        rstd2, murstd2 = _ln_rows(nc, pools, pmu2, pmsq2)
        while fillers:
            fillers.pop(0)()
        qT2 = _q_correct(nc, pools, qpre2, rstd2, murstd2, sq_e, bq_e)

        # FFN LN stats + W1 projection interleave into the cross-attention
        # output tail: each residual tile feeds its stats / W1 matmuls as
        # soon as it lands, so the LN row chain finishes under PE cover.
        sbp = pools["scratch"]
        fst = {}

        def ffn_post(k, ybf_k):
            if k == 0:
                # allocate AFTER all attention psum tiles so pool rotation
                # dependencies stay in emission order
                fst["pmu3"] = pools["ps_s"].tile([128, NQ], F32, name="pmu3", tag="spsum")
                fst["pmsq3"] = pools["ps_s"].tile([128, NQ], F32, name="pmsq3", tag="spsum")
                fst["pre"] = [pools["ps_a"].tile([128, NQ], F32, name="ffnpre", tag="apsum")
                              for _ in range(DI // 128)]
            sq3 = sbp.tile([128, NQ], BF16, name="sq3", tag=f"sq_{k}", bufs=1)
            nc.vector.tensor_mul(out=sq3, in0=ybf_k, in1=ybf_k)
            nc.tensor.matmul(fst["pmu3"], lhsT=aux["invD"], rhs=ybf_k,
                             start=(k == 0), stop=(k == KT - 1))
            nc.tensor.matmul(fst["pmsq3"], lhsT=aux["invD"], rhs=sq3,
                             start=(k == 0), stop=(k == KT - 1))
            for m in range(DI // 128):
                nc.tensor.matmul(fst["pre"][m], lhsT=w1t[k][:, m * 128:(m + 1) * 128],
                                 rhs=ybf_k, start=(k == 0), stop=(k == KT - 1))

        x2, x2bf = _attention(nc, pools, qT2, x1, kT2, vv2, wfc_e,
                              [NQ] * LT, None, post_m=ffn_post)

        # ---- FFN ----
        pre_ps = fst["pre"]
        rstd3, murstd3 = _ln_rows(nc, pools, fst["pmu3"], fst["pmsq3"])
        relu = []
        for m in range(DI // 128):
            bb = pools["scratch"].tile([128, NQ], BF16, tag="lnb", bufs=6)
            nc.vector.tensor_scalar(out=bb, in0=murstd3, scalar1=s1c[m],
                                    scalar2=b1c[m], op0=AluOpType.mult,
                                    op1=AluOpType.subtract)
            t2 = pools["scratch"].tile([128, NQ], F32, tag="xntmp", bufs=2)
            nc.vector.tensor_mul(out=t2, in0=pre_ps[m], in1=rstd3)
            t3 = pools["scratch"].tile([128, NQ], F32, tag="xntmp", bufs=2)
            nc.vector.tensor_sub(out=t3, in0=t2, in1=bb)
            r = act.tile([128, NQ], BF16, name="relu", tag=f"relu_{m}")
            nc.vector.tensor_scalar_max(out=r, in0=t3, scalar1=0.0)
            relu.append(r)
        obig = act.tile([128, KT, NQ], F32, name="obig", tag="obig")
        for m in range(KT):
            p = pools["ps_proj"].tile([128, NQ], F32, tag="proj")
            for k in range(DI // 128):
                nc.tensor.matmul(p, lhsT=w2t[k][:, m * 128:(m + 1) * 128], rhs=relu[k],
                                 start=(k == 0), stop=(k == DI // 128 - 1))
            tmp = pools["scratch"].tile([128, NQ], F32, tag="xntmp", bufs=2)
            nc.scalar.add(out=tmp, in_=p, add=b2t[m])
            nc.vector.tensor_add(out=obig[:, m, :], in0=tmp, in1=x2[m])
            if m == 1:
                nc.sync.dma_start(
                    out=out_d.rearrange("(k p) j -> p k j", p=128)[:, 0:2, :],
                    in_=obig[:, 0:2, :])
        nc.sync.dma_start(out=out_d.rearrange("(k p) j -> p k j", p=128)[:, 2:4, :],
                          in_=obig[:, 2:4, :])

    _split_multi_waits(nc)
    return nc


# ---------------------------------------------------------------------------
# Host side
# ---------------------------------------------------------------------------

_CACHE = {}


def _slot_blocks(half):
    return [7, 5, 2, 0] if half == 0 else [6, 4, 3, 1]


def _qrows(half):
    return np.concatenate([np.arange(b * 128, (b + 1) * 128) for b in _slot_blocks(half)])


def kernel(**inputs):
    dec = np.asarray(inputs["dec_input"], np.float32)
    enc = np.asarray(inputs["enc_output"], np.float32)
    maskin = np.asarray(inputs["slf_attn_mask"])
    mask2d = (maskin[0] != 0)  # [Lq, Lk] bool

    bf = ml_dtypes.bfloat16

    def wT(x):
        x = np.asarray(x, np.float32).reshape(-1, x.shape[-1])
        return np.ascontiguousarray(x.T).astype(bf)

    def wq_fold(wkey, gkey, bkey):
        wflat = np.asarray(inputs[wkey], np.float32).reshape(-1, D)     # [hdk, D]
        g = np.asarray(inputs[gkey], np.float32)
        b = np.asarray(inputs[bkey], np.float32)
        wg = wflat * g[None, :]
        s = wg.sum(axis=1)                                              # [hdk]
        bq = wflat @ b
        return np.ascontiguousarray(wg.T).astype(bf), s, bq

    wq_s, s_s, b_s = wq_fold("slf_Wq", "slf_ln_g", "slf_ln_b")
    wq_e, s_e, b_e = wq_fold("enc_Wq", "enc_ln_g", "enc_ln_b")
    w_t = {
        "wq_s": wq_s, "wk_s": wT(inputs["slf_Wk"]),
        "wv_s": wT(inputs["slf_Wv"]),
        "wfc_s": np.ascontiguousarray(np.asarray(inputs["slf_Wfc"], np.float32).T).astype(bf),
        "wq_e": wq_e, "wk_e": wT(inputs["enc_Wk"]),
        "wv_e": wT(inputs["enc_Wv"]),
        "wfc_e": np.ascontiguousarray(np.asarray(inputs["enc_Wfc"], np.float32).T).astype(bf),
    }
    w1f = np.asarray(inputs["ffn_W1"], np.float32)          # [DI, D]
    g_f = np.asarray(inputs["ffn_ln_g"], np.float32)
    b_lnf = np.asarray(inputs["ffn_ln_b"], np.float32)
    w1g = w1f * g_f[None, :]
    w1 = np.ascontiguousarray(w1g.T).astype(bf)
    s1 = w1g.sum(axis=1)                                    # [DI]
    b1eff = w1f @ b_lnf + np.asarray(inputs["ffn_b1"], np.float32)
    w2 = np.ascontiguousarray(np.asarray(inputs["ffn_W2"], np.float32).T).astype(bf)

    # per-partition scalar columns: [128, 20]
    def cols(v, n):
        return np.stack([v[m * 128:(m + 1) * 128] for m in range(n)], axis=1)

    scols = np.concatenate([
        cols(s_s, KT), cols(b_s, KT), cols(s_e, KT), cols(b_e, KT),
        cols(s1, DI // 128), cols(b1eff, DI // 128),
    ], axis=1).astype(np.float32)                           # [128, 20]

    b1 = np.asarray(inputs["ffn_b1"], np.float32)
    b2 = np.asarray(inputs["ffn_b2"], np.float32)
    bvecs = np.stack(
        [b1[0:128], b1[128:256]] + [b2[m * 128:(m + 1) * 128] for m in range(KT)],
        axis=1).astype(np.float32)                          # [128, 6]

    # per-core mask tiles + uniform prefix schedule
    half_masks = []
    nt_sched = [0] * LT
    for half in range(2):
        rowsq = _qrows(half)
        m = mask2d[rowsq, :]
        tiles = np.zeros((LT, 128, NQ), np.float32)
        for t in range(LT):
            blk = m[:, t * 128:(t + 1) * 128]
            tiles[t] = blk.T.astype(np.float32)
            for s in range(MQ):
                if blk[s * 128:(s + 1) * 128, :].any():
                    nt_sched[t] = max(nt_sched[t], (s + 1) * 128)
        half_masks.append(tiles)
    nt_sched[0] = NQ
    nt_key = tuple(nt_sched)

    layout = pair_layout(nt_sched)
    core_masks = []
    for half in range(2):
        packed = np.zeros((LT // 2, 128, 2 * NQ), np.float32)
        for pi, (regions, _w) in enumerate(layout):
            for (t, off, n) in regions:
                packed[pi][:, off:off + n] = half_masks[half][t][:, 0:n]
        core_masks.append(packed.astype(bf))

    if nt_key not in _CACHE:
        _CACHE[nt_key] = build_program(list(nt_key))
    nc = _CACHE[nt_key]

    in_maps = []
    for c in range(NCORES):
        b, half = divmod(c, 2)
        rowsq = _qrows(half)
        decT = np.ascontiguousarray(dec[b].T)          # [D, L] f32
        encT = np.ascontiguousarray(enc[b].T)
        xq32 = np.ascontiguousarray(decT[:, rowsq])
        in_maps.append({
            "xq32": xq32,
            "xq16": xq32.astype(bf),
            "xkv": decT.astype(bf),
            "xenc": encT.astype(bf),
            "mask": core_masks[half],
            "w1": w1, "w2": w2,
            "scols": scols,
            "bvecs": bvecs,
            **w_t,
        })

    from concourse.bass_utils import run_bass_kernel_spmd

    res = run_bass_kernel_spmd(nc, in_maps, core_ids=list(range(NCORES)))
    globals()["_LAST_RESULT"] = res

    out = np.empty((B, L, D), np.float32)
    for c in range(NCORES):
        b, half = divmod(c, 2)
        out[b, _qrows(half), :] = res.results[c]["out"].T
    return out


# revision 17
# speedup vs baseline: 1.5818x; 1.0252x over previous
"""Trainium2 Bass kernel for nn_DecoderLayer (self-attn -> cross-attn -> FFN).

Distribution: 8 NeuronCores = 4 batches x 2 causal-balanced sequence halves.
Core (b, h) processes 512 query rows of batch b through the entire layer:
half 0 owns row blocks {7,5,2,0} (x128), half 1 owns {6,4,3,1} - descending
block order makes the causally-visible k-tiles of each query chunk a prefix,
so one uniform SPMD program serves both halves (masks arrive as data).
No inter-core communication: each core computes K/V projections for the full
sequence itself.

On-device layout is fully transposed (feature dim on SBUF partitions); the
host pre-transposes inputs/weights and re-transposes the output.  All matmuls
run in bf16 (f32 PSUM accumulation); the residual path stays f32.

LayerNorm row statistics are computed directly in broadcast form: the stats
matmul uses an all-1/D [128,128] stationary tile, so mean / mean-square land
replicated across all 128 partitions and the rsqrt Newton iteration runs as
full-width DVE ops - no DMA reshape round trips, no PE broadcast matmuls.
Softmax denominators ride the AV matmul: V carries 64 extra all-ones columns,
so PSUM rows 64:128 hold the denominator replicated 64-wide and the per-head
normalization is one DVE reciprocal + one multiply.
"""

import contextlib

import numpy as np
import ml_dtypes

import concourse.bass as bass
import concourse.mybir as mybir
import concourse.tile as tile

B, L, D, H, DK, DI = 4, 1024, 512, 8, 64, 256
TEMP = float(DK) ** 0.5
NCORES = 8
KT = D // 128   # 4 feature tiles
LT = L // 128   # 8 sequence tiles
NQ = 512        # own query columns per core
MQ = NQ // 128  # 4 query slots

F32 = mybir.dt.float32
BF16 = mybir.dt.bfloat16
AF = mybir.ActivationFunctionType
ALU = None


# ---------------------------------------------------------------------------
# Workarounds for the walrus build in this container: at most ONE semaphore
# wait per instruction.  Split extra waits onto same-engine NoOps.
# ---------------------------------------------------------------------------

def _patch_drain_split():
    from concourse.vector_clock import ScopedClock

    if getattr(tile.TileContext, "_drain_split_patched", False):
        return

    def _drain_and_barrier(self, tick_clock, wait_clock):
        gc = tick_clock.global_clock
        for idx in range(len(gc)):
            t = gc[idx]
            if t <= 0:
                continue
            req = ScopedClock()
            req.require_at_least(None, idx, t)
            nop_inst = self.nc.sync.nop(nofuse=True, hint=f"drain_wait_{idx}")
            wait_clock.add_sem_waits(nop_inst.ins, req)
        self.nc.sync.drain()
        self.nc.all_engine_barrier()
        assert self.sems is not None
        popped = self.nc._tile_sem_poison_stack.pop()
        assert popped is self._sem_poison
        self.nc.clear_and_free_semaphores(list(self.sems.allocated().values()))
        self.nc.all_engine_barrier()

    tile.TileContext._drain_and_barrier = _drain_and_barrier
    tile.TileContext._drain_split_patched = True


def _split_multi_waits(nc, max_waits=1):
    import bass_rust

    ctr = 0
    for fn in nc.m.functions:
        for blk in fn.blocks:
            changed = False
            new_insts = []
            for inst in blk.instructions:
                si = inst.sync_info
                if si is not None and si.on_wait and len(si.on_wait) > max_waits:
                    waits = list(si.on_wait)
                    for w in waits[:-max_waits]:
                        ctr += 1
                        nop = mybir.InstNoOp(name=f"WSPLIT-{ctr}", ins=[], outs=[])
                        nop.engine = inst.engine
                        nop.sync_info = bass_rust.SyncInfo(on_wait=[w], on_update=[])
                        new_insts.append(nop)
                    inst.sync_info = bass_rust.SyncInfo(
                        on_wait=waits[-max_waits:], on_update=list(si.on_update or [])
                    )
                    changed = True
                new_insts.append(inst)
            if changed:
                blk.instructions = new_insts
    return ctr


_patch_drain_split()


# ---------------------------------------------------------------------------
# Device program
# ---------------------------------------------------------------------------

def _ln_stats(nc, pools, xbf, aux):
    """mu / mean-square stats, broadcast across all 128 partitions."""
    sb, ps_s = pools["scratch"], pools["ps_s"]
    invD = aux["invD"]
    pmu = ps_s.tile([128, NQ], F32, name="pmu", tag="spsum")
    pmsq = ps_s.tile([128, NQ], F32, name="pmsq", tag="spsum")
    sq = [None] * KT
    for k in range(KT):
        sq[k] = sb.tile([128, NQ], BF16, name="sq", tag=f"sq_{k}", bufs=1)
        nc.vector.tensor_mul(out=sq[k], in0=xbf[k], in1=xbf[k])
    for k in range(KT):
        nc.tensor.matmul(pmu, lhsT=invD, rhs=xbf[k], start=(k == 0), stop=(k == KT - 1))
    for k in range(KT):
        nc.tensor.matmul(pmsq, lhsT=invD, rhs=sq[k], start=(k == 0), stop=(k == KT - 1))
    return pmu, pmsq


def _ln_rows(nc, pools, pmu, pmsq):
    """rstd / mu*rstd as broadcast [128,NQ] bf16 tiles.  All serial steps on
    ACT: square, then rsqrt(var) = exp(-0.5*ln(var)) - ln and exp are
    resident together in the natural_log_exp table set.  DVE does only the
    variance subtract and the mu*rstd multiply."""
    sb = pools["scratch"]
    musq = sb.tile([128, NQ], F32, tag="lnt", bufs=2)
    nc.scalar.activation(out=musq, in_=pmu, func=AF.Square, scale=1.0)
    var = sb.tile([128, NQ], F32, tag="lnt", bufs=2)
    nc.vector.tensor_sub(out=var, in0=pmsq, in1=musq)
    lg = sb.tile([128, NQ], F32, tag="lnt", bufs=2)
    nc.scalar.activation(out=lg, in_=var, func=AF.Ln, scale=1.0)
    y = sb.tile([128, NQ], BF16, tag="lnb", bufs=6)
    nc.scalar.activation(out=y, in_=lg, func=AF.Exp, scale=-0.5)
    mr = sb.tile([128, NQ], BF16, tag="lnb", bufs=6)
    with nc.allow_low_precision(reason="layernorm rows in bf16"):
        nc.vector.tensor_mul(out=mr, in0=pmu, in1=y)
    return y, mr


def _q_correct(nc, pools, pre, rstd_b, murstd_b, s_cols, b_cols):
    """qT[j,l] = rstd[l]*pre[j,l] - (murstd[l]*s[j] - b[j]) - all DVE."""
    from concourse.alu_op_type import AluOpType
    sb = pools["scratch"]
    act = pools["act"]
    qT = [None] * KT
    for m in range(KT):
        bb = sb.tile([128, NQ], BF16, tag="lnb", bufs=6)
        nc.vector.tensor_scalar(out=bb, in0=murstd_b, scalar1=s_cols[m],
                                scalar2=b_cols[m], op0=AluOpType.mult,
                                op1=AluOpType.subtract)
        t2 = sb.tile([128, NQ], BF16, tag="lnb", bufs=6)
        with nc.allow_low_precision(reason="q in bf16 anyway"):
            nc.vector.tensor_mul(out=t2, in0=pre[m], in1=rstd_b)
        qT[m] = act.tile([128, NQ], BF16, name="qT", tag=f"qT_{m}")
        nc.vector.tensor_sub(out=qT[m], in0=t2, in1=bb)
    return qT


def pair_layout(nt_sched):
    """Pack two k-tiles' score columns into one [128,1024] PSUM tile without
    any matmul crossing a 2KB PSUM bank boundary."""
    out = []
    for pi in range(LT // 2):
        t0, t1 = 2 * pi, 2 * pi + 1
        n0, n1 = nt_sched[t0], nt_sched[t1]
        regions = []
        off = 0
        if n0 > 0:
            regions.append((t0, 0, n0))
            off = n0
        if n1 > 0:
            o1 = off if off + n1 <= 512 else 512
            regions.append((t1, o1, n1))
            off = o1 + n1
        out.append((regions, off))
    return out


def _contig_runs(regions):
    """Group regions into maximal contiguous column runs [(start, end, [t...])]."""
    runs = []
    for (t, off, n) in regions:
        if runs and runs[-1][1] == off:
            runs[-1][1] = off + n
            runs[-1][2].append(t)
        else:
            runs.append([off, off + n, [t]])
    return runs


def _xkv_load(nc, pools, xkv):
    act = pools["act"]
    big = act.tile([128, KT, L], BF16, name="xkv_sb", tag="xkv_sb", bufs=2)
    xr = xkv.rearrange("(k p) j -> p k j", p=128)
    nc.sync.dma_start(out=big[:, :, 0:L // 2], in_=xr[:, :, 0:L // 2])
    nc.sync.dma_start(out=big[:, :, L // 2:L], in_=xr[:, :, L // 2:L])
    return [big[:, k, :] for k in range(KT)]


def _k_project(nc, pools, xkv_sb, wk):
    act = pools["act"]
    ps_proj = pools["ps_proj"]
    kT = [None] * KT
    for m in range(KT):
        kT[m] = act.tile([128, L], BF16, name="kT", tag=f"kT_{m}", bufs=2)
        for c in range(L // 512):
            p = ps_proj.tile([128, 512], F32, tag="proj")
            for k in range(KT):
                nc.tensor.matmul(p, lhsT=wk[k][:, m * 128:(m + 1) * 128],
                                 rhs=xkv_sb[k][:, c * 512:(c + 1) * 512],
                                 start=(k == 0), stop=(k == KT - 1))
            if (m + c) % 2 == 0:
                nc.vector.tensor_copy(out=kT[m][:, c * 512:(c + 1) * 512], in_=p)
            else:
                nc.scalar.copy(out=kT[m][:, c * 512:(c + 1) * 512], in_=p)
    return kT


def _v_fill(nc, pools, xkv_sb, wv, vv, t):
    """One V-projection PSUM group into vv[t][:, :, 0:64] (evac on ACT)."""
    ps_proj = pools["ps_proj"]
    p = ps_proj.tile([128, 512], F32, tag="proj")
    for k in range(KT):
        nc.tensor.matmul(p, lhsT=xkv_sb[k][:, t * 128:(t + 1) * 128], rhs=wv[k],
                         start=(k == 0), stop=(k == KT - 1))
    if t % 2 == 0:
        nc.scalar.copy(out=vv[t][:, :, 0:64],
                       in_=p.rearrange("p (h v) -> p h v", h=H))
    else:
        nc.vector.tensor_copy(out=vv[t][:, :, 0:64],
                              in_=p.rearrange("p (h v) -> p h v", h=H))


def _kv_fillers(nc, pools, xkv_sb, wk, wv, vv2):
    """Closure list computing next-layer K/V one PSUM group at a time."""
    act = pools["act"]
    ps_proj = pools["ps_proj"]
    kT = [act.tile([128, L], BF16, name="kT", tag=f"kT_{m}", bufs=2)
          for m in range(KT)]
    fillers = []

    def kf(m, c):
        def f():
            p = ps_proj.tile([128, 512], F32, tag="proj")
            for k in range(KT):
                nc.tensor.matmul(p, lhsT=wk[k][:, m * 128:(m + 1) * 128],
                                 rhs=xkv_sb[k][:, c * 512:(c + 1) * 512],
                                 start=(k == 0), stop=(k == KT - 1))
            if (m + c) % 2 == 0:
                nc.vector.tensor_copy(out=kT[m][:, c * 512:(c + 1) * 512], in_=p)
            else:
                nc.scalar.copy(out=kT[m][:, c * 512:(c + 1) * 512], in_=p)
        return f

    for m in range(KT):
        for c in range(L // 512):
            fillers.append(kf(m, c))
    for t in range(LT):
        fillers.append(lambda t=t: _v_fill(nc, pools, xkv_sb, wv, vv2, t))
    return kT, fillers


def _q_pre(nc, pools, wq, xin):
    sb = pools["scratch"]
    ps_proj = pools["ps_proj"]
    pre = [None] * KT
    for m in range(KT):
        p = ps_proj.tile([128, NQ], F32, tag="proj")
        for k in range(KT):
            nc.tensor.matmul(p, lhsT=wq[k][:, m * 128:(m + 1) * 128], rhs=xin[k],
                             start=(k == 0), stop=(k == KT - 1))
        pre[m] = sb.tile([128, NQ], F32, name="qpre", tag=f"qpre_{m}", bufs=1)
        if m % 2 == 0:
            nc.vector.tensor_copy(out=pre[m], in_=p)
        else:
            nc.scalar.copy(out=pre[m], in_=p)
    return pre


def _attention(nc, pools, qT, x32, kT, vv, wfc, nt_sched, mask_tiles, fillers=None,
               post_m=None):
    """One MHA block.  a_ps rows 0:64 = per-head AV, rows 64:128 = softmax
    denominator broadcast 64-wide (via the ones half of vv).  Per-head tail
    is one DVE reciprocal + one multiply, issued one head late."""
    sb = pools["scratch"]
    act = pools["act"]
    ps_proj = pools["ps_proj"]
    ps_s = pools["ps_s"]
    ps_a = pools["ps_a"]
    ppool = pools["ppool"]

    attnT = [act.tile([128, NQ], BF16, name="attnT", tag=f"attnT_{m}") for m in range(KT)]

    def make_tail(h, a_ps):
        # 1/den = exp(-ln(den)) on ACT (both tables resident); DVE only
        # does the final multiply.
        def f():
            lg = sb.tile([64, NQ], F32, name="lg", tag="lg", bufs=2)
            nc.scalar.activation(out=lg, in_=a_ps[64:128, :], func=AF.Ln, scale=1.0)
            rb = sb.tile([64, NQ], BF16, name="rb", tag="rb", bufs=2)
            nc.scalar.activation(out=rb, in_=lg, func=AF.Exp, scale=-1.0)
            nc.vector.tensor_mul(
                out=attnT[h // 2][(h % 2) * 64:(h % 2) * 64 + 64, :],
                in0=a_ps[0:64, :], in1=rb,
            )
        return f

    layout = pair_layout(nt_sched)
    last_pair = max(pi for pi, (rg, _) in enumerate(layout) if rg)
    o1 = [None] * KT

    def o_first(m):
        def f():
            p = ps_proj.tile([128, NQ], F32, tag="proj")
            for k in range(2):
                nc.tensor.matmul(p, lhsT=wfc[k][:, m * 128:(m + 1) * 128],
                                 rhs=attnT[k], start=(k == 0), stop=(k == 1))
            o1[m] = sb.tile([128, NQ], F32, name="o1", tag=f"o1_{m}", bufs=1)
            if m % 2 == 0:
                nc.vector.tensor_copy(out=o1[m], in_=p)
            else:
                nc.scalar.copy(out=o1[m], in_=p)
        return f

    late = [o_first(m) for m in range(KT)]

    tails = []
    for h in range(H):
        kh = kT[h // 2][(h % 2) * 64:(h % 2) * 64 + 64, :]
        qh = qT[h // 2][(h % 2) * 64:(h % 2) * 64 + 64, :]
        a_ps = ps_a.tile([128, NQ], F32, tag="apsum")
        for pi, (regions, width) in enumerate(layout):
            if not regions:
                continue
            if pi == 1 and tails:
                tails.pop(0)()
            s_ps = ps_s.tile([128, 2 * NQ], F32, tag="spsum")
            p_sb = ppool.tile([128, 2 * NQ], BF16, tag="p")
            for (t, off, n) in regions:
                nc.tensor.matmul(s_ps[:, off:off + n],
                                 lhsT=kh[:, t * 128:(t + 1) * 128],
                                 rhs=qh[:, 0:n], start=True, stop=True,
                                 skip_group_check=True)
            for (lo, hi, _ts) in _contig_runs(regions):
                nc.scalar.activation(out=p_sb[:, lo:hi], in_=s_ps[:, lo:hi],
                                     func=AF.Exp, scale=1.0 / TEMP)
                if mask_tiles is not None:
                    nc.vector.tensor_mul(out=p_sb[:, lo:hi], in0=p_sb[:, lo:hi],
                                         in1=mask_tiles[pi][:, lo:hi])
            for ri, (t, off, n) in enumerate(regions):
                nc.tensor.matmul(a_ps[:, 0:n], lhsT=vv[t][:, h, :],
                                 rhs=p_sb[:, off:off + n],
                                 start=(pi == 0 and ri == 0),
                                 stop=(pi == last_pair and ri == len(regions) - 1),
                                 skip_group_check=True)
            if fillers and 1 <= h <= 2:
                fillers.pop(0)()
            if late and h >= 5 and pi % 2 == 1:
                late.pop(0)()
        tails.append(make_tail(h, a_ps))
    while late:
        late.pop(0)()

    # Flush remaining head tails with PE filler between them, then the
    # second halves (k=2,3) of the output projection.
    if fillers:
        fillers.pop(0)()
    while tails:
        tails.pop(0)()
        if fillers:
            fillers.pop(0)()

    y32 = [None] * KT
    ybf = [None] * KT

    def o_second(m, p):
        t = sb.tile([128, NQ], F32, tag="xntmp", bufs=2)
        nc.vector.tensor_add(out=t, in0=p, in1=o1[m])
        y = act.tile([128, NQ], F32, name="resid", tag=f"resid_{m}", bufs=2)
        nc.vector.tensor_add(out=y, in0=t, in1=x32[m])
        yb = act.tile([128, NQ], BF16, name="xbf", tag=f"xbf_{m}")
        nc.vector.tensor_copy(out=yb, in_=y)
        return y, yb

    for m in range(KT):
        p = ps_proj.tile([128, NQ], F32, tag="proj")
        for k in range(2, KT):
            nc.tensor.matmul(p, lhsT=wfc[k][:, m * 128:(m + 1) * 128], rhs=attnT[k],
                             start=(k == 2), stop=(k == KT - 1))
        y32[m], ybf[m] = o_second(m, p)
        if post_m is not None:
            post_m(m, ybf[m])
    return y32, ybf


def build_program(nt_sched):
    from concourse.alu_op_type import AluOpType
    nc = bass.Bass("TRN2", target_bir_lowering=False, debug=False)

    def din(name, shape, dt=BF16):
        return nc.dram_tensor(name, shape, dt, kind="ExternalInput").ap()

    xq16 = din("xq16", [D, NQ])
    xq32 = din("xq32", [D, NQ], F32)
    xkv = din("xkv", [D, L])
    xenc = din("xenc", [D, L])
    maskd = din("mask", [LT // 2, 128, 2 * NQ])
    w = {n: din(n, [D, D]) for n in
         ["wq_s", "wk_s", "wv_s", "wfc_s", "wq_e", "wk_e", "wv_e", "wfc_e"]}
    w1 = din("w1", [D, DI])
    w2 = din("w2", [DI, D])
    scolsd = din("scols", [128, 20], F32)
    bvecs = din("bvecs", [128, DI // 128 + KT], F32)
    out_d = nc.dram_tensor("out", [D, NQ], F32, kind="ExternalOutput").ap()

    with tile.TileContext(nc) as tc, contextlib.ExitStack() as ctx:
        pools = {
            "const": ctx.enter_context(tc.tile_pool(name="const", bufs=1)),
            "wpool": ctx.enter_context(tc.tile_pool(name="wpool", bufs=1)),
            "xpool": ctx.enter_context(tc.tile_pool(name="xpool", bufs=1)),
            "act": ctx.enter_context(tc.tile_pool(name="act", bufs=1)),
            "scratch": ctx.enter_context(tc.tile_pool(name="scratch", bufs=2)),
            "ppool": ctx.enter_context(tc.tile_pool(name="ppool", bufs=4)),
            "ps_proj": ctx.enter_context(tc.tile_pool(name="ps_proj", bufs=2, space="PSUM")),
            "ps_s": ctx.enter_context(tc.tile_pool(name="ps_s", bufs=2, space="PSUM")),
            "ps_a": ctx.enter_context(tc.tile_pool(name="ps_a", bufs=2, space="PSUM")),
        }
        const = pools["const"]
        xpool = pools["xpool"]
        wpool = pools["wpool"]
        act = pools["act"]

        aux = {}
        aux["invD"] = const.tile([128, 128], BF16, name="invD")
        nc.vector.memset(aux["invD"], 1.0 / D)

        def load_w(name, tag):
            big = wpool.tile([128, KT, D], BF16, name=tag, tag=tag)
            nc.sync.dma_start(out=big, in_=w[name].rearrange("(k p) j -> p k j", p=128))
            return [big[:, k, :] for k in range(KT)]

        # earliest DMAs in need order: stats1 needs xq16; K-proj needs wk+xkv
        x16big = xpool.tile([128, KT, NQ], BF16, name="xq16t", tag="xq16t")
        nc.sync.dma_start(out=x16big, in_=xq16.rearrange("(k p) j -> p k j", p=128))
        xbf = [x16big[:, m, :] for m in range(KT)]
        wk = load_w("wk_s", "wk")
        xkv1 = _xkv_load(nc, pools, xkv)
        wq = load_w("wq_s", "wq")
        wv = load_w("wv_s", "wv")
        mbig = xpool.tile([128, LT // 2, 2 * NQ], BF16, name="maskt", tag="maskt")
        nc.sync.dma_start(out=mbig, in_=maskd.rearrange("t p j -> p t j"))
        mask_tiles = [mbig[:, t, :] for t in range(LT // 2)]
        x32big = xpool.tile([128, KT, NQ], F32, name="xq32t", tag="xq32t")
        nc.sync.dma_start(out=x32big, in_=xq32.rearrange("(k p) j -> p k j", p=128))
        x32 = [x32big[:, m, :] for m in range(KT)]
        wfc = load_w("wfc_s", "wfc")
        scols_t = const.tile([128, 20], F32, name="scols_t")
        nc.sync.dma_start(out=scols_t, in_=scolsd)
        sq_s = [scols_t[:, m:m + 1] for m in range(KT)]
        bq_s = [scols_t[:, 4 + m:5 + m] for m in range(KT)]
        sq_e = [scols_t[:, 8 + m:9 + m] for m in range(KT)]
        bq_e = [scols_t[:, 12 + m:13 + m] for m in range(KT)]
        s1c = [scols_t[:, 16 + m:17 + m] for m in range(DI // 128)]
        b1c = [scols_t[:, 18 + m:19 + m] for m in range(DI // 128)]
        bcols = const.tile([128, DI // 128 + KT], F32, name="bcols")
        nc.sync.dma_start(out=bcols, in_=bvecs)
        b2t = [bcols[:, DI // 128 + m:DI // 128 + m + 1] for m in range(KT)]
        xkv2 = _xkv_load(nc, pools, xenc)
        wq_e = load_w("wq_e", "wq2")
        wk_e = load_w("wk_e", "wk2")
        wv_e = load_w("wv_e", "wv2")
        wfc_e = load_w("wfc_e", "wfc2")
        w1big = wpool.tile([128, KT, DI], BF16, name="w1t", tag="w1t")
        nc.sync.dma_start(out=w1big, in_=w1.rearrange("(k p) j -> p k j", p=128))
        w1t = [w1big[:, k, :] for k in range(KT)]
        w2big = wpool.tile([128, DI // 128, D], BF16, name="w2t", tag="w2t")
        nc.sync.dma_start(out=w2big, in_=w2.rearrange("(k p) j -> p k j", p=128))
        w2t = [w2big[:, k, :] for k in range(DI // 128)]

        # V tiles with the ones half pre-set (softmax denominator broadcast)
        vv1 = [act.tile([128, H, 128], BF16, name="vv1", tag=f"vv1_{t}")
               for t in range(LT)]
        vv2 = [act.tile([128, H, 128], BF16, name="vv2", tag=f"vv2_{t}")
               for t in range(LT)]
        for t in range(LT):
            nc.gpsimd.memset(vv1[t][:, :, 64:128], 1.0)
            nc.gpsimd.memset(vv2[t][:, :, 64:128], 1.0)

        # ---- self attention ----
        pmu1, pmsq1 = _ln_stats(nc, pools, xbf, aux)
        kT1 = _k_project(nc, pools, xkv1, wk)
        rstd1, murstd1 = _ln_rows(nc, pools, pmu1, pmsq1)
        qpre1 = _q_pre(nc, pools, wq, xbf)
        for t in range(LT):
            _v_fill(nc, pools, xkv1, wv, vv1, t)
        qT1 = _q_correct(nc, pools, qpre1, rstd1, murstd1, sq_s, bq_s)
        kT2, fillers = _kv_fillers(nc, pools, xkv2, wk_e, wv_e, vv2)
        x1, x1bf = _attention(nc, pools, qT1, x32, kT1, vv1, wfc,
                              nt_sched, mask_tiles, fillers=fillers)

        # ---- cross attention ----
        pmu2, pmsq2 = _ln_stats(nc, pools, x1bf, aux)
        qpre2 = _q_pre(nc, pools, wq_e, x1bf)
        rstd2, murstd2 = _ln_rows(nc, pools, pmu2, pmsq2)
        while fillers:
            fillers.pop(0)()
        qT2 = _q_correct(nc, pools, qpre2, rstd2, murstd2, sq_e, bq_e)

        # FFN LN stats + W1 projection interleave into the cross-attention
        # output tail: each residual tile feeds its stats / W1 matmuls as
        # soon as it lands, so the LN row chain finishes under PE cover.
        sbp = pools["scratch"]
        fst = {}

        def ffn_post(k, ybf_k):
            if k == 0:
                # allocate AFTER all attention psum tiles so pool rotation
                # dependencies stay in emission order
                fst["pmu3"] = pools["ps_s"].tile([128, NQ], F32, name="pmu3", tag="spsum")
                fst["pmsq3"] = pools["ps_s"].tile([128, NQ], F32, name="pmsq3", tag="spsum")
                fst["pre"] = [pools["ps_a"].tile([128, NQ], F32, name="ffnpre", tag="apsum")
                              for _ in range(DI // 128)]
            sq3 = sbp.tile([128, NQ], BF16, name="sq3", tag=f"sq_{k}", bufs=1)
            nc.vector.tensor_mul(out=sq3, in0=ybf_k, in1=ybf_k)
            nc.tensor.matmul(fst["pmu3"], lhsT=aux["invD"], rhs=ybf_k,
                             start=(k == 0), stop=(k == KT - 1))
            nc.tensor.matmul(fst["pmsq3"], lhsT=aux["invD"], rhs=sq3,
                             start=(k == 0), stop=(k == KT - 1))
            for m in range(DI // 128):
                nc.tensor.matmul(fst["pre"][m], lhsT=w1t[k][:, m * 128:(m + 1) * 128],
                                 rhs=ybf_k, start=(k == 0), stop=(k == KT - 1))

        x2, x2bf = _attention(nc, pools, qT2, x1, kT2, vv2, wfc_e,
                              [NQ] * LT, None, post_m=ffn_post)

        # ---- FFN ----
        pre_ps = fst["pre"]
        rstd3, murstd3 = _ln_rows(nc, pools, fst["pmu3"], fst["pmsq3"])
        relu = []
        for m in range(DI // 128):
            bb = pools["scratch"].tile([128, NQ], BF16, tag="lnb", bufs=6)
            nc.vector.tensor_scalar(out=bb, in0=murstd3, scalar1=s1c[m],
                                    scalar2=b1c[m], op0=AluOpType.mult,
                                    op1=AluOpType.subtract)
            t2 = pools["scratch"].tile([128, NQ], F32, tag="xntmp", bufs=2)
            nc.vector.tensor_mul(out=t2, in0=pre_ps[m], in1=rstd3)
            t3 = pools["scratch"].tile([128, NQ], F32, tag="xntmp", bufs=2)
            nc.vector.tensor_sub(out=t3, in0=t2, in1=bb)
            r = act.tile([128, NQ], BF16, name="relu", tag=f"relu_{m}")
            nc.vector.tensor_scalar_max(out=r, in0=t3, scalar1=0.0)
            relu.append(r)
        obig = act.tile([128, KT, NQ], F32, name="obig", tag="obig")
        for m in range(KT):
            p = pools["ps_proj"].tile([128, NQ], F32, tag="proj")
            for k in range(DI // 128):
                nc.tensor.matmul(p, lhsT=w2t[k][:, m * 128:(m + 1) * 128], rhs=relu[k],
                                 start=(k == 0), stop=(k == DI // 128 - 1))
            tmp = pools["scratch"].tile([128, NQ], F32, tag="xntmp", bufs=2)
            nc.scalar.add(out=tmp, in_=p, add=b2t[m])
            nc.vector.tensor_add(out=obig[:, m, :], in0=tmp, in1=x2[m])
            if m == 1:
                nc.sync.dma_start(
                    out=out_d.rearrange("(k p) j -> p k j", p=128)[:, 0:2, :],
                    in_=obig[:, 0:2, :])
        nc.sync.dma_start(out=out_d.rearrange("(k p) j -> p k j", p=128)[:, 2:4, :],
                          in_=obig[:, 2:4, :])

    _split_multi_waits(nc)
    return nc


# ---------------------------------------------------------------------------
# Host side
# ---------------------------------------------------------------------------

_CACHE = {}


def _slot_blocks(half):
    return [7, 5, 2, 0] if half == 0 else [6, 4, 3, 1]


def _qrows(half):
    return np.concatenate([np.arange(b * 128, (b + 1) * 128) for b in _slot_blocks(half)])


def kernel(**inputs):
    dec = np.asarray(inputs["dec_input"], np.float32)
    enc = np.asarray(inputs["enc_output"], np.float32)
    maskin = np.asarray(inputs["slf_attn_mask"])
    mask2d = (maskin[0] != 0)  # [Lq, Lk] bool

    bf = ml_dtypes.bfloat16

    def wT(x):
        x = np.asarray(x, np.float32).reshape(-1, x.shape[-1])
        return np.ascontiguousarray(x.T).astype(bf)

    def wq_fold(wkey, gkey, bkey):
        wflat = np.asarray(inputs[wkey], np.float32).reshape(-1, D)     # [hdk, D]
        g = np.asarray(inputs[gkey], np.float32)
        b = np.asarray(inputs[bkey], np.float32)
        wg = wflat * g[None, :]
        s = wg.sum(axis=1)                                              # [hdk]
        bq = wflat @ b
        return np.ascontiguousarray(wg.T).astype(bf), s, bq

    wq_s, s_s, b_s = wq_fold("slf_Wq", "slf_ln_g", "slf_ln_b")
    wq_e, s_e, b_e = wq_fold("enc_Wq", "enc_ln_g", "enc_ln_b")
    w_t = {
        "wq_s": wq_s, "wk_s": wT(inputs["slf_Wk"]),
        "wv_s": wT(inputs["slf_Wv"]),
        "wfc_s": np.ascontiguousarray(np.asarray(inputs["slf_Wfc"], np.float32).T).astype(bf),
        "wq_e": wq_e, "wk_e": wT(inputs["enc_Wk"]),
        "wv_e": wT(inputs["enc_Wv"]),
        "wfc_e": np.ascontiguousarray(np.asarray(inputs["enc_Wfc"], np.float32).T).astype(bf),
    }
    w1f = np.asarray(inputs["ffn_W1"], np.float32)          # [DI, D]
    g_f = np.asarray(inputs["ffn_ln_g"], np.float32)
    b_lnf = np.asarray(inputs["ffn_ln_b"], np.float32)
    w1g = w1f * g_f[None, :]
    w1 = np.ascontiguousarray(w1g.T).astype(bf)
    s1 = w1g.sum(axis=1)                                    # [DI]
    b1eff = w1f @ b_lnf + np.asarray(inputs["ffn_b1"], np.float32)
    w2 = np.ascontiguousarray(np.asarray(inputs["ffn_W2"], np.float32).T).astype(bf)

    # per-partition scalar columns: [128, 20]
    def cols(v, n):
        return np.stack([v[m * 128:(m + 1) * 128] for m in range(n)], axis=1)

    scols = np.concatenate([
        cols(s_s, KT), cols(b_s, KT), cols(s_e, KT), cols(b_e, KT),
        cols(s1, DI // 128), cols(b1eff, DI // 128),
    ], axis=1).astype(np.float32)                           # [128, 20]

    b1 = np.asarray(inputs["ffn_b1"], np.float32)
    b2 = np.asarray(inputs["ffn_b2"], np.float32)
    bvecs = np.stack(
        [b1[0:128], b1[128:256]] + [b2[m * 128:(m + 1) * 128] for m in range(KT)],
        axis=1).astype(np.float32)                          # [128, 6]

    # per-core mask tiles + uniform prefix schedule
    half_masks = []
    nt_sched = [0] * LT
    for half in range(2):
        rowsq = _qrows(half)
        m = mask2d[rowsq, :]
        tiles = np.zeros((LT, 128, NQ), np.float32)
        for t in range(LT):
            blk = m[:, t * 128:(t + 1) * 128]
            tiles[t] = blk.T.astype(np.float32)
            for s in range(MQ):
                if blk[s * 128:(s + 1) * 128, :].any():
                    nt_sched[t] = max(nt_sched[t], (s + 1) * 128)
        half_masks.append(tiles)
    nt_sched[0] = NQ
    nt_key = tuple(nt_sched)

    layout = pair_layout(nt_sched)
    core_masks = []
    for half in range(2):
        packed = np.zeros((LT // 2, 128, 2 * NQ), np.float32)
        for pi, (regions, _w) in enumerate(layout):
            for (t, off, n) in regions:
                packed[pi][:, off:off + n] = half_masks[half][t][:, 0:n]
        core_masks.append(packed.astype(bf))

    if nt_key not in _CACHE:
        _CACHE[nt_key] = build_program(list(nt_key))
    nc = _CACHE[nt_key]

    in_maps = []
    for c in range(NCORES):
        b, half = divmod(c, 2)
        rowsq = _qrows(half)
        decT = np.ascontiguousarray(dec[b].T)          # [D, L] f32
        encT = np.ascontiguousarray(enc[b].T)
        xq32 = np.ascontiguousarray(decT[:, rowsq])
        in_maps.append({
            "xq32": xq32,
            "xq16": xq32.astype(bf),
            "xkv": decT.astype(bf),
            "xenc": encT.astype(bf),
            "mask": core_masks[half],
            "w1": w1, "w2": w2,
            "scols": scols,
            "bvecs": bvecs,
            **w_t,
        })

    from concourse.bass_utils import run_bass_kernel_spmd

    res = run_bass_kernel_spmd(nc, in_maps, core_ids=list(range(NCORES)))
    globals()["_LAST_RESULT"] = res

    out = np.empty((B, L, D), np.float32)
    for c in range(NCORES):
        b, half = divmod(c, 2)
        out[b, _qrows(half), :] = res.results[c]["out"].T
    return out
